# revision 45
# baseline (speedup 1.0000x reference)
"""Trainium2 Bass kernel for nn_DepthCueRectification_Sp.

Data-parallel over batch: 8 batch elements -> 8 NeuronCores (SPMD).

Host precomputes (exact, f32): pos softmax (posn = SA*g*softmax(pos_logits)),
the CLS output row (tiny 1x1536 MLP), and all weight/layout prep. The device
handles only the N=1024 patch tokens - no padding, no CLS special cases.

Per-core pipeline (D=768, N=1024):
  tT    = U @ xb.T                  (bf16)
  yUT   = U @ yb.T                  (bf16)   [algebra: logits_k =
                                     (|S_k|*t) @ (y@U.T).T]
  tsT_k = |S_k|-scaled copies of tT (ACT per-partition scale)
  logits_k -> exp (no max-sub, fused row-sum) -> attn_k = 256*attn (bf16)
  patch_k += posn (host-exact pos term, bf16)
  entropy: Ln on ACT; fused mult+row-sum via DVE scalar_tensor_tensor
  accum_out; routing compares raw accums; heat = 2e/(1+e) via one exp
  dka (selected attn, f32r) -> PE transpose -> acT fp8 pairs
  y_outT = fp8 DoubleRow (ybp pairs @ acT pairs) -> yf8 = 16*y_out.T fp8
  MLP W1: x-half bf16 (xtb @ 32*W1a) + y-half fp8 DR (yf8 @ 2*W1b) = 32*h
  gel   = gelu(psh/32 + b1) -> fp8 pairs
  MLP W2: fp8 DoubleRow (gel pairs @ 32*W2 pairs) = 32*xp'
  out   = x + heat*(xp' + b2)

The act-table dict is patched so Exp and Ln resolve to the combined
natural_log_exp_and_others set (avoids per-iteration table reloads).
tensor_tensor_reduce and 16-bit PE transposes hard-crash the exec unit
on this toolchain and are not used.
"""

import os
import sys

if "/opt/trn_rl_repo" not in sys.path:
    sys.path.insert(0, "/opt/trn_rl_repo")

import numpy as np
import ml_dtypes
from scipy.special import erf

import concourse.bass as bass
import concourse.bass_utils as _bu
import concourse.mybir as mybir
import concourse.tile as tile
from concourse import bacc
from concourse.bass_utils import run_bass_kernel_spmd
from concourse.hw_specs import get_activation_tables
from concourse.masks import make_identity

B, N, D, DFF = 8, 1024, 768, 3072
ND = D // 128          # 6
NB = N // 128          # 8
NF = DFF // 128        # 24
AF = mybir.ActivationFunctionType
ALU = mybir.AluOpType
dt = mybir.dt
DR = mybir.MatmulPerfMode.DoubleRow

NODR = bool(int(os.environ.get("K_NODR", "0")))      # disable DoubleRow
TR32R = bool(int(os.environ.get("K_TR32R", "0")))    # f32r PE transposes

SCALE = float(D) ** -0.5
SA = 256.0             # attn scale (fp8 headroom)
SY = 16.0              # y_full scale in yf8
SW1X = 32.0            # W1 x-half scale (bf16)  == SW1Y*SY
SW1Y = 2.0             # W1 y-half scale (fp8)
SW2 = 32.0             # W2 scale (fp8)
LN256 = float(np.log(SA))

_prog_cache = {}


def _patch_act_tables(arch):
    """Make natural_log_exp_and_others the only provider of Exp/Ln so the
    compiler's table-load pass keeps one table across the attention loop.
    Mutates the functools-cached dict in place (names/ids unchanged)."""
    tabs = get_activation_tables(arch)
    keep = "natural_log_exp_and_others"
    if keep not in tabs:
        return
    for name, s in tabs.items():
        if name == keep:
            continue
        s.discard(AF.Exp)
        s.discard(AF.Ln)


def _build(g, ht, pt):
    omg = 1.0 - g
    f8 = dt.float8e4
    bf = dt.bfloat16
    f32 = dt.float32
    f32t = dt.float32r if TR32R else dt.float32

    nc = bacc.Bacc("TRN2", target_bir_lowering=False, debug=False, num_devices=8)
    _patch_act_tables(nc.m.arch)

    def mm_dr(out, l3, r3, start, stop):
        if not NODR:
            nc.tensor.matmul(out, l3, r3, start=start, stop=stop, perf_mode=DR)
        else:
            nc.tensor.matmul(out, l3[:, 0], r3[:, 0], start=start, stop=False)
            nc.tensor.matmul(out, l3[:, 1], r3[:, 1], start=False, stop=stop)

    # ---- DRAM params ----
    xtb_d = nc.declare_dram_parameter("xtb", [128, ND, N], bf, isOutput=False)
    yt_d = nc.declare_dram_parameter("yt", [128, ND, N], bf, isOutput=False)
    ybp_d = nc.declare_dram_parameter("ybp", [128, 4, 2, D], f8, isOutput=False)
    utb_d = nc.declare_dram_parameter("utb", [128, ND, ND, 128], bf, isOutput=False)
    posn_d = nc.declare_dram_parameter("posn", [128, NB, N], bf, isOutput=False)
    w1p_d = nc.declare_dram_parameter("w1p", [128, 3, NF, 2, 128], f8, isOutput=False)
    w1x_d = nc.declare_dram_parameter("w1x", [128, ND, NF, 128], bf, isOutput=False)
    w2r_d = nc.declare_dram_parameter("w2r", [128, 12, 2, D], f8, isOutput=False)
    b1t_d = nc.declare_dram_parameter("b1t", [128, NF], f32, isOutput=False)
    b2r_d = nc.declare_dram_parameter("b2r", [1, D], bf, isOutput=False)
    s12_d = nc.declare_dram_parameter("s12", [128, 2, ND], f32, isOutput=False)
    xnat_d = nc.declare_dram_parameter("xnat", [N, D], f32, isOutput=False)
    out_d = nc.declare_dram_parameter("out", [N, D], f32, isOutput=True)
    hmbuf = nc.dram_tensor("hmbuf", [N, 1], f32)

    with tile.TileContext(nc) as tc:
        with tc.tile_pool(name="p0", bufs=1) as P0:
            # ---- persistent tiles ----
            w1p = P0.tile([128, 3, NF, 2, 128], f8, tag="w1p", name="w1p")
            w1x = P0.tile([128, ND, NF, 128], bf, tag="w1x", name="w1x")
            xtb = P0.tile([128, ND, N], bf, tag="xtb", name="xtb")
            yf8 = P0.tile([128, 3, 2, N], f8, tag="yf8", name="yf8")
            posn = P0.tile([128, NB, N], bf, tag="posn", name="posn")
            b2r = P0.tile([1, D], bf, tag="b2r", name="b2r")
            b1t = P0.tile([128, NF], f32, tag="b1t", name="b1t")
            s12 = P0.tile([128, 2, ND], f32, tag="s12", name="s12")
            identf = P0.tile([128, 128], f32, tag="identf", name="identf")
            identm = P0.tile([128, 128], bf, tag="identm", name="identm")
            onesr = P0.tile([1, 128], bf, tag="onesr", name="onesr")
            epsb = P0.tile([128, 1], f32, tag="epsb", name="epsb")
            zerop = P0.tile([128, 1], f32, tag="zerop", name="zerop")
            hbias = P0.tile([128, 1], f32, tag="hbias", name="hbias")

            # ---- gpsimd queue: s12 (phase-1 tail dep) + small inits ----
            nc.gpsimd.dma_start(s12[:], s12_d[:])
            make_identity(nc, identf[:])
            make_identity(nc, identm[:])
            nc.gpsimd.memset(epsb[:], SA * 1e-8)
            nc.gpsimd.memset(hbias[:], -ht * LN256)
            nc.gpsimd.memset(zerop[:], 0.0)
            nc.gpsimd.memset(onesr[:], 1.0)

            # ---- PE warmup: ramp the tensor engine to full clock while
            # the input DMAs stream (identm has no DMA dependency). ----
            with tc.tile_pool(name="pwu", bufs=1, space="PSUM") as PWU:
                wps = PWU.tile([128, 128], f32, tag="wps", name="wps")
                for _ in range(24):
                    nc.tensor.matmul(wps[:], identm[:], identm[:],
                                     start=True, stop=True)

            with tc.tile_pool(name="pa2", bufs=1) as PA2:
                acT = PA2.tile([128, 4, 2, N], f8, tag="acT", name="acT")
                ybp = PA2.tile([128, 4, 2, D], f8, tag="ybp", name="ybp")

                with tc.tile_pool(name="pa1", bufs=1) as PA1:
                    yUT = PA1.tile([128, ND, N], bf, tag="yUT", name="yUT")
                    ts0 = PA1.tile([128, ND, N], bf, tag="ts0", name="ts0")
                    ts1 = PA1.tile([128, ND, N], bf, tag="ts1", name="ts1")

                    # ---------- phase 1: tT, yUT ----------
                    with tc.tile_pool(name="p1", bufs=1) as P1, \
                         tc.tile_pool(name="ps1", bufs=2, space="PSUM") as PS1:
                        utb = P1.tile([128, ND, ND, 128], bf, tag="utb", name="utb")
                        yt = P1.tile([128, ND, N], bf, tag="yt", name="yt")
                        # phase-1-critical inputs first on their queues
                        for d in range(ND):
                            nc.scalar.dma_start(utb[:, d], utb_d[:, d])
                        for k in range(ND):
                            nc.sync.dma_start(xtb[:, k], xtb_d[:, k])
                        for k in range(ND):
                            nc.gpsimd.dma_start(yt[:, k], yt_d[:, k])
                        # first two posn blocks up front; rest stream
                        # just-in-time inside the phase-2 loop
                        for nb in range(2):
                            nc.sync.dma_start(posn[:, nb], posn_d[:, nb])

                        for d in range(ND):
                            ps = PS1.tile([128, N], f32, tag="psA", name="psA")
                            for k in range(ND):
                                for h in range(2):
                                    nc.tensor.matmul(
                                        ps[:, 512 * h : 512 * h + 512],
                                        utb[:, d, k],
                                        xtb[:, k, 512 * h : 512 * h + 512],
                                        start=(k == 0), stop=(k == ND - 1),
                                    )
                            nc.vector.tensor_scalar_mul(
                                ts0[:, d, :], ps[:], s12[:, 0, d : d + 1])
                            nc.scalar.mul(ts1[:, d, :], ps[:], s12[:, 1, d : d + 1])
                        for d in range(ND):
                            ps = PS1.tile([128, N], f32, tag="psA", name="psA")
                            for k in range(ND):
                                for h in range(2):
                                    nc.tensor.matmul(
                                        ps[:, 512 * h : 512 * h + 512],
                                        utb[:, d, k],
                                        yt[:, k, 512 * h : 512 * h + 512],
                                        start=(k == 0), stop=(k == ND - 1),
                                    )
                            nc.scalar.copy(yUT[:, d, :], ps[:])

                    # ---- phase 2: attention, entropy, routing ----
                    # Every engine queue is in-order, so the per-block chain
                    # exp -> combine -> ln -> mult -> reduce -> route -> dka
                    # -> transpose is software-pipelined three deep: loop
                    # step i emits S0(i), S1(i-1), S2(i-2). Each engine then
                    # always has ready work at its queue head.
                    with tc.tile_pool(name="pat", bufs=4) as PT, \
                         tc.tile_pool(name="plk", bufs=3) as LK, \
                         tc.tile_pool(name="pdk", bufs=4) as DK, \
                         tc.tile_pool(name="psm", bufs=8) as SM, \
                         tc.tile_pool(name="psl", bufs=3, space="PSUM") as PSL, \
                         tc.tile_pool(name="pstp", bufs=2, space="PSUM") as PST:

                        # big phase-3/4 weights stream on the sync queue
                        # during phase 2 (it only carries posn + heat stores)
                        w14_loads = (
                            [lambda i=i: nc.sync.dma_start(ybp[:, i], ybp_d[:, i])
                             for i in range(4)]
                            + [lambda: nc.sync.dma_start(b1t[:], b1t_d[:])]
                            + [lambda i=i: nc.sync.dma_start(w1p[:, i], w1p_d[:, i])
                               for i in range(3)]
                            + [lambda i=i: nc.sync.dma_start(w1x[:, i], w1x_d[:, i])
                               for i in range(ND)]
                        )

                        st_ = {}

                        def s0(nb):
                            # logits GEMMs, exp with fused row-sum, 1/esum
                            if 2 + nb < NB:
                                nc.sync.dma_start(posn[:, 2 + nb],
                                                  posn_d[:, 2 + nb])
                            for li in range(2 * nb, min(2 * nb + 2,
                                                        len(w14_loads))):
                                w14_loads[li]()
                            pk = PT.tile([128, 2, N], bf, tag="pk", name="pk")
                            rks = []
                            for k2 in range(2):
                                tsk = ts0 if k2 == 0 else ts1
                                psl = PSL.tile([128, N], f32, tag="psl",
                                               name="psl")
                                for e in range(ND):
                                    for h in range(2):
                                        nc.tensor.matmul(
                                            psl[:, 512 * h : 512 * h + 512],
                                            tsk[:, e, 128 * nb : 128 * nb + 128],
                                            yUT[:, e, 512 * h : 512 * h + 512],
                                            start=(e == 0), stop=(e == ND - 1),
                                        )
                                esum = SM.tile([128, 1], f32, tag="esum",
                                               name="esum")
                                nc.scalar.activation(pk[:, k2, :], psl[:],
                                                     AF.Exp, bias=zerop[:],
                                                     scale=SCALE,
                                                     accum_out=esum[:])
                                rkf = SM.tile([128, 1], f32, tag="rkf",
                                              name="rkf")
                                nc.vector.reciprocal(rkf[:], esum[:])
                                rk = SM.tile([128, 1], bf, tag="rk", name="rk")
                                nc.vector.tensor_scalar_mul(rk[:], rkf[:],
                                                            SA * omg)
                                rks.append(rk)
                            st_[("pk", nb)] = pk
                            st_[("rk", nb)] = rks

                        def s1(nb):
                            # pos combine, ln, entropy product
                            pk = st_[("pk", nb)]
                            rks = st_.pop(("rk", nb))
                            lnk = LK.tile([128, 2, N], bf, tag="lnk",
                                          name="lnk")
                            for k2 in range(2):
                                patch = pk[:, k2, :]
                                nc.vector.scalar_tensor_tensor(
                                    patch, patch, rks[k2][:], posn[:, nb, :],
                                    ALU.mult, ALU.add)
                                nc.scalar.activation(lnk[:, k2, :], patch,
                                                     AF.Ln, bias=epsb[:])
                                nc.gpsimd.tensor_mul(lnk[:, k2, :],
                                                     lnk[:, k2, :], patch)
                            d01 = DK.tile([128, N], bf, tag="d01", name="d01")
                            nc.gpsimd.tensor_sub(d01[:], pk[:, 0, :],
                                                 pk[:, 1, :])
                            st_[("lnk", nb)] = lnk
                            st_[("d01", nb)] = d01

                        def s2(nb):
                            # entropy reduce, routing, heat, dka, transposes
                            r0 = 128 * nb
                            pk = st_.pop(("pk", nb))
                            lnk = st_.pop(("lnk", nb))
                            d01 = st_.pop(("d01", nb))
                            accr = SM.tile([128, 2], f32, tag="accr",
                                           name="accr")
                            nc.vector.tensor_reduce(
                                accr[:], lnk[:], axis=mybir.AxisListType.X,
                                op=ALU.add)
                            # route0 iff H0<=H1 iff accr0>=accr1
                            rsel = SM.tile([128, 1], bf, tag="rsel",
                                           name="rsel")
                            nc.vector.tensor_tensor(rsel[:], accr[:, 0:1],
                                                    accr[:, 1:2], ALU.is_ge)
                            amax = SM.tile([128, 1], f32, tag="amax",
                                           name="amax")
                            nc.vector.tensor_tensor(amax[:], accr[:, 0:1],
                                                    accr[:, 1:2], ALU.max)
                            # e = exp(-ht*H_sel) = exp(ht/256*amax - ht*ln256)
                            ee = SM.tile([128, 1], f32, tag="ee", name="ee")
                            nc.scalar.activation(ee[:], amax[:], AF.Exp,
                                                 scale=ht / SA, bias=hbias[:])
                            ep1 = SM.tile([128, 1], f32, tag="ep1", name="ep1")
                            nc.vector.tensor_scalar_add(ep1[:], ee[:], 1.0)
                            rcp = SM.tile([128, 1], f32, tag="rcp", name="rcp")
                            nc.vector.reciprocal(rcp[:], ep1[:])
                            # heat pre-scaled by 1/SW2 (W2 epilogue reads it
                            # as the only scale on the pso accumulator)
                            heat = SM.tile([128, 1], f32, tag="heat",
                                           name="heat")
                            nc.vector.scalar_tensor_tensor(
                                heat[:], ee[:], 2.0 / SW2, rcp[:],
                                ALU.mult, ALU.mult)
                            nc.sync.dma_start(hmbuf[r0 : r0 + 128, 0:1],
                                              heat[:])
                            dka = DK.tile([128, N], f32t, tag="dka",
                                          name="dka")
                            nc.vector.scalar_tensor_tensor(
                                dka[:], d01[:], rsel[:], pk[:, 1, :],
                                ALU.mult, ALU.add)
                            # 8 transposes into two 512-wide PSUM tiles, one
                            # batched fp8 copy per group of 4
                            for grp in range(2):
                                pst = PST.tile([128, 512], f32t, tag="pst",
                                               name="pst")
                                for q in range(4):
                                    mb = 4 * grp + q
                                    nc.tensor.transpose(
                                        pst[:, 128 * q : 128 * q + 128],
                                        dka[:, 128 * mb : 128 * mb + 128],
                                        identf[:])
                                dst = acT[:, 2 * grp : 2 * grp + 2, :,
                                          r0 : r0 + 128]
                                nc.scalar.copy(dst, pst[:])

                        for i in range(NB + 2):
                            if i < NB:
                                s0(i)
                            if 1 <= i <= NB:
                                s1(i - 1)
                            if i >= 2:
                                s2(i - 2)

                # ---------- phase 3: y_outT (fp8 DoubleRow) -> yf8 ----------
                with tc.tile_pool(name="psy", bufs=2, space="PSUM") as PSY:
                    for d in range(ND):
                        psy = PSY.tile([128, N], f32, tag="psy", name="psy")
                        for mbp in range(4):
                            for h in range(2):
                                mm_dr(
                                    psy[:, 512 * h : 512 * h + 512],
                                    ybp[:, mbp, :, 128 * d : 128 * d + 128],
                                    acT[:, mbp, :, 512 * h : 512 * h + 512],
                                    (mbp == 0), (mbp == 3),
                                )
                        if d % 2 == 0:
                            nc.scalar.mul(yf8[:, d // 2, d % 2, :],
                                          psy[:], SY / SA)
                        else:
                            nc.vector.tensor_scalar_mul(
                                yf8[:, d // 2, d % 2, :], psy[:], SY / SA)

            # ---------- phase 4: MLP ----------
            with tc.tile_pool(name="pg", bufs=1) as PG:
                w2r = PG.tile([128, 12, 2, D], f8, tag="w2r", name="w2r")
                gel = PG.tile([128, 12, 2, N], f8, tag="gel", name="gel")
                nc.sync.dma_start(b2r[:], b2r_d[:])
                for q in range(4):
                    nc.sync.dma_start(w2r[:, 3 * q : 3 * q + 3],
                                      w2r_d[:, 3 * q : 3 * q + 3])

                chunksA = [(0, 512), (512, 512)]
                with tc.tile_pool(name="psh", bufs=2, space="PSUM") as PSH:
                    for f in range(NF):
                        psh = PSH.tile([128, N], f32, tag="psh", name="psh")
                        for c in range(ND):
                            for (s0, wd) in chunksA:
                                nc.tensor.matmul(
                                    psh[:, s0 : s0 + wd],
                                    w1x[:, c, f],
                                    xtb[:, c, s0 : s0 + wd],
                                    start=(c == 0), stop=False,
                                )
                        for yp in range(3):
                            for (s0, wd) in chunksA:
                                mm_dr(
                                    psh[:, s0 : s0 + wd],
                                    w1p[:, yp, f],
                                    yf8[:, yp, :, s0 : s0 + wd],
                                    False, (yp == 2),
                                )
                        nc.scalar.activation(gel[:, f // 2, f % 2, :], psh[:],
                                             AF.Gelu, bias=b1t[:, f : f + 1],
                                             scale=1.0 / SW1X)

                with tc.tile_pool(name="p5", bufs=3) as P5, \
                     tc.tile_pool(name="pso", bufs=2, space="PSUM") as PSO:
                    chunksB = [(512, D - 512), (0, 512)]
                    for tb in range(NB):
                        r0 = 128 * tb
                        xn = P5.tile([128, D], f32, tag="xn", name="xn")
                        nc.sync.dma_start(xn[:], xnat_d[r0 : r0 + 128, :])
                        hmc = P5.tile([128, 1], f32, tag="hmc", name="hmc")
                        nc.sync.dma_start(hmc[:], hmbuf[r0 : r0 + 128, 0:1])
                        pso = PSO.tile([128, D], f32, tag="pso", name="pso")
                        for fp in range(12):
                            for (s0, wd) in chunksB:
                                mm_dr(
                                    pso[:, s0 : s0 + wd],
                                    gel[:, fp, :, r0 : r0 + 128],
                                    w2r[:, fp, :, s0 : s0 + wd],
                                    (fp == 0), False,
                                )
                        # rank-1 b2 bias fold: pso += ones.T @ (SW2*b2)
                        for (s0, wd) in chunksB:
                            nc.tensor.matmul(
                                pso[:, s0 : s0 + wd],
                                onesr[0:1, 0:128],
                                b2r[0:1, s0 : s0 + wd],
                                start=False, stop=True,
                            )
                        # single drain op: out = pso*(heat/SW2) + x
                        ot = P5.tile([128, D], f32, tag="ot", name="ot")
                        nc.vector.scalar_tensor_tensor(
                            ot[:], pso[:], hmc[:], xn[:],
                            ALU.mult, ALU.add)
                        nc.sync.dma_start(out_d[r0 : r0 + 128, :], ot[:])

    nc.compile()
    return nc


def _get_prog(g, ht, pt):
    key = (round(float(g), 9), round(float(ht), 9), round(float(pt), 9))
    if key not in _prog_cache:
        _prog_cache[key] = _build(*key)
    return _prog_cache[key]


def kernel(x, y, coords, U, S1, S2, gating, h_temp, p_temp, pos_emb, W1, b1, W2, b2):
    x = np.asarray(x, dtype=np.float32)
    y = np.asarray(y, dtype=np.float32)
    coords = np.asarray(coords, dtype=np.float32)
    U = np.asarray(U, dtype=np.float32)
    bf16 = ml_dtypes.bfloat16
    f8 = ml_dtypes.float8_e4m3

    g = float(1.0 / (1.0 + np.exp(-float(np.asarray(gating)))))
    ht = float(np.asarray(h_temp))
    pt = abs(float(np.asarray(p_temp)))
    nc = _get_prog(g, ht, pt)

    def q8(a):
        return np.clip(a, -240.0, 240.0).astype(f8)

    # ---- shared (replicated) host prep ----
    UT = np.ascontiguousarray(U.T)
    utb = np.ascontiguousarray(
        UT.reshape(ND, 128, ND, 128).transpose(1, 2, 0, 3)).astype(bf16)
    s12 = np.ascontiguousarray(np.stack(
        [np.abs(np.asarray(S1, np.float32)).reshape(ND, 128).T,
         np.abs(np.asarray(S2, np.float32)).reshape(ND, 128).T], axis=1))
    # pos softmax on host (exact):
    #   posn[i, j] = SA*g*softmax_j(-pt * sum_c coords[i,j,c]*pe[i,c])
    pe_f = np.asarray(pos_emb, np.float32)[:, :, 0]            # [N, 6]
    pl = -pt * np.einsum("ijc,ic->ij", coords, pe_f)           # [N, N]
    pl -= pl.max(axis=1, keepdims=True)
    pexp = np.exp(pl)
    psm = pexp / pexp.sum(axis=1, keepdims=True)
    posn = np.ascontiguousarray(
        (SA * g * psm).reshape(NB, 128, N).transpose(1, 0, 2)).astype(bf16)
    W1 = np.asarray(W1, np.float32)
    W1a, W1b = W1[:D], W1[D:]
    w1x = np.ascontiguousarray(
        (SW1X * W1a).reshape(ND, 128, NF, 128).transpose(1, 0, 2, 3)).astype(bf16)
    w1p = q8((SW1Y * W1b).reshape(3, 2, 128, NF, 128).transpose(2, 0, 3, 1, 4))
    W2 = np.asarray(W2, np.float32)
    w2r = q8((SW2 * W2).reshape(12, 2, 128, D).transpose(2, 0, 1, 3))
    b1 = np.asarray(b1, np.float32)
    b2 = np.asarray(b2, np.float32)
    b1t = np.ascontiguousarray(b1.reshape(NF, 128).T)
    b2r = np.ascontiguousarray((SW2 * b2).reshape(1, D)).astype(bf16)

    shared = {"utb": utb, "s12": s12, "posn": posn,
              "w1x": w1x, "w1p": w1p, "w2r": w2r, "b1t": b1t, "b2r": b2r}

    in_maps = []
    cls_rows = []
    for b in range(B):
        xb = x[b, 1:]
        yb = y[b, 1:]
        xtb = np.ascontiguousarray(
            xb.T.reshape(ND, 128, N).transpose(1, 0, 2)).astype(bf16)
        yt = np.ascontiguousarray(
            yb.T.reshape(ND, 128, N).transpose(1, 0, 2)).astype(bf16)
        ybp = q8(yb.reshape(4, 2, 128, D).transpose(2, 0, 1, 3))
        # CLS output row on host (exact f32 1x1536 MLP; no heat scaling)
        h0 = np.concatenate([x[b, 0], y[b, 0]]) @ W1 + b1
        xp0 = (h0 * 0.5 * (1.0 + erf(h0 / np.sqrt(2.0)))) @ W2 + b2
        cls_rows.append(x[b, 0] + xp0)
        m = dict(shared)
        m["xtb"] = xtb
        m["yt"] = yt
        m["ybp"] = ybp
        m["xnat"] = np.ascontiguousarray(xb)
        in_maps.append(m)

    res = run_bass_kernel_spmd(nc, in_maps, list(range(B)))
    out = np.empty((B, N + 1, D), np.float32)
    for b in range(B):
        out[b, 0] = cls_rows[b]
        out[b, 1:] = res.results[b]["out"]
    return out


if __name__ == "__main__":
    import time
    sys.path.insert(0, "/root/problem")
    from reference import setup_inputs, reference

    inp = {k: np.asarray(v) for k, v in setup_inputs().items()}
    t0 = time.time()
    got = kernel(**inp)
    print("kernel wall:", time.time() - t0)
    exp = np.asarray(reference(**inp))
    d = np.abs(got - exp)
    print("absmax_rel:", d.max() / np.abs(exp).max())
    print("rms_rel:", np.sqrt((d ** 2).mean()) / np.sqrt((exp ** 2).mean()))


# revision 48
# speedup vs baseline: 1.0779x; 1.0779x over previous
"""Trainium2 Bass kernel for nn_DepthCueRectification_Sp.

Data-parallel over batch: 8 batch elements -> 8 NeuronCores (SPMD).

Host precomputes (exact, f32): pos softmax (posn = SA*g*softmax(pos_logits)),
the CLS output row (tiny 1x1536 MLP), and all weight/layout prep. The device
handles only the N=1024 patch tokens - no padding, no CLS special cases.

Per-core pipeline (D=768, N=1024):
  tT    = U @ xb.T                  (bf16)
  yUT   = U @ yb.T                  (bf16)   [algebra: logits_k =
                                     (|S_k|*t) @ (y@U.T).T]
  tsT_k = |S_k|-scaled copies of tT (ACT per-partition scale)
  logits_k -> exp (no max-sub, fused row-sum) -> attn_k = 256*attn (bf16)
  patch_k += posn (host-exact pos term, bf16)
  entropy: Ln on ACT; fused mult+row-sum via DVE scalar_tensor_tensor
  accum_out; routing compares raw accums; heat = 2e/(1+e) via one exp
  dka (selected attn, f32r) -> PE transpose -> acT fp8 pairs
  y_outT = fp8 DoubleRow (ybp pairs @ acT pairs) -> yf8 = 16*y_out.T fp8
  MLP W1: x-half bf16 (xtb @ 32*W1a) + y-half fp8 DR (yf8 @ 2*W1b) = 32*h
  gel   = gelu(psh/32 + b1) -> fp8 pairs
  MLP W2: fp8 DoubleRow (gel pairs @ 32*W2 pairs) = 32*xp'
  out   = x + heat*(xp' + b2)

The act-table dict is patched so Exp and Ln resolve to the combined
natural_log_exp_and_others set (avoids per-iteration table reloads).
tensor_tensor_reduce and 16-bit PE transposes hard-crash the exec unit
on this toolchain and are not used.
"""

import os
import sys

if "/opt/trn_rl_repo" not in sys.path:
    sys.path.insert(0, "/opt/trn_rl_repo")

import numpy as np
import ml_dtypes
from scipy.special import erf

import concourse.bass as bass
import concourse.bass_utils as _bu
import concourse.mybir as mybir
import concourse.tile as tile
from concourse import bacc
from concourse.bass_utils import run_bass_kernel_spmd
from concourse.hw_specs import get_activation_tables
from concourse.masks import make_identity

B, N, D, DFF = 8, 1024, 768, 3072
ND = D // 128          # 6
NB = N // 128          # 8
NF = DFF // 128        # 24
AF = mybir.ActivationFunctionType
ALU = mybir.AluOpType
dt = mybir.dt
DR = mybir.MatmulPerfMode.DoubleRow

NODR = bool(int(os.environ.get("K_NODR", "0")))      # disable DoubleRow
TR32R = bool(int(os.environ.get("K_TR32R", "0")))    # f32r PE transposes

SCALE = float(D) ** -0.5
SA = 256.0             # attn scale (fp8 headroom)
SY = 16.0              # y_full scale in yf8
SW1X = 32.0            # W1 x-half scale (bf16)  == SW1Y*SY
SW1Y = 2.0             # W1 y-half scale (fp8)
SW2 = 32.0             # W2 scale (fp8)
LN256 = float(np.log(SA))

_prog_cache = {}


def _patch_act_tables(arch):
    """Make natural_log_exp_and_others the only provider of Exp/Ln so the
    compiler's table-load pass keeps one table across the attention loop.
    Mutates the functools-cached dict in place (names/ids unchanged)."""
    tabs = get_activation_tables(arch)
    keep = "natural_log_exp_and_others"
    if keep not in tabs:
        return
    for name, s in tabs.items():
        if name == keep:
            continue
        s.discard(AF.Exp)
        s.discard(AF.Ln)


def _build(g, ht, pt):
    omg = 1.0 - g
    f8 = dt.float8e4
    bf = dt.bfloat16
    f32 = dt.float32
    f32t = dt.float32r if TR32R else dt.float32

    nc = bacc.Bacc("TRN2", target_bir_lowering=False, debug=False, num_devices=8)
    _patch_act_tables(nc.m.arch)

    def mm_dr(out, l3, r3, start, stop):
        if not NODR:
            nc.tensor.matmul(out, l3, r3, start=start, stop=stop, perf_mode=DR)
        else:
            nc.tensor.matmul(out, l3[:, 0], r3[:, 0], start=start, stop=False)
            nc.tensor.matmul(out, l3[:, 1], r3[:, 1], start=False, stop=stop)

    # ---- DRAM params ----
    xtb_d = nc.declare_dram_parameter("xtb", [128, ND, N], bf, isOutput=False)
    yt_d = nc.declare_dram_parameter("yt", [128, ND, N], bf, isOutput=False)
    ybp_d = nc.declare_dram_parameter("ybp", [128, 4, 2, D], f8, isOutput=False)
    utb_d = nc.declare_dram_parameter("utb", [128, ND, ND, 128], bf, isOutput=False)
    posn_d = nc.declare_dram_parameter("posn", [128, NB, N], bf, isOutput=False)
    w1p_d = nc.declare_dram_parameter("w1p", [128, 3, NF, 2, 128], f8, isOutput=False)
    w1x_d = nc.declare_dram_parameter("w1x", [128, ND, NF, 128], bf, isOutput=False)
    w2r_d = nc.declare_dram_parameter("w2r", [128, 12, 2, D], f8, isOutput=False)
    b1t_d = nc.declare_dram_parameter("b1t", [128, NF], f32, isOutput=False)
    b2r_d = nc.declare_dram_parameter("b2r", [1, D], bf, isOutput=False)
    s12_d = nc.declare_dram_parameter("s12", [128, 2, ND], f32, isOutput=False)
    xnat_d = nc.declare_dram_parameter("xnat", [N, D], f32, isOutput=False)
    out_d = nc.declare_dram_parameter("out", [N, D], f32, isOutput=True)
    hmbuf = nc.dram_tensor("hmbuf", [N, 1], f32)

    with tile.TileContext(nc) as tc:
        with tc.tile_pool(name="p0", bufs=1) as P0:
            # ---- persistent tiles ----
            w1p = P0.tile([128, 3, NF, 2, 128], f8, tag="w1p", name="w1p")
            w1x = P0.tile([128, ND, NF, 128], bf, tag="w1x", name="w1x")
            xtb = P0.tile([128, ND, N], bf, tag="xtb", name="xtb")
            yf8 = P0.tile([128, 3, 2, N], f8, tag="yf8", name="yf8")
            posn = P0.tile([128, NB, N], bf, tag="posn", name="posn")
            b2r = P0.tile([1, D], bf, tag="b2r", name="b2r")
            b1t = P0.tile([128, NF], f32, tag="b1t", name="b1t")
            s12 = P0.tile([128, 2, ND], f32, tag="s12", name="s12")
            identf = P0.tile([128, 128], f32, tag="identf", name="identf")
            identm = P0.tile([128, 128], bf, tag="identm", name="identm")
            onesr = P0.tile([1, 128], bf, tag="onesr", name="onesr")
            epsb = P0.tile([128, 1], f32, tag="epsb", name="epsb")
            zerop = P0.tile([128, 1], f32, tag="zerop", name="zerop")
            hbias = P0.tile([128, 1], f32, tag="hbias", name="hbias")

            # ---- gpsimd queue: s12 (phase-1 tail dep) + small inits ----
            nc.gpsimd.dma_start(s12[:], s12_d[:])
            make_identity(nc, identf[:])
            make_identity(nc, identm[:])
            nc.gpsimd.memset(epsb[:], SA * 1e-8)
            nc.gpsimd.memset(hbias[:], -ht * LN256)
            nc.gpsimd.memset(zerop[:], 0.0)
            nc.gpsimd.memset(onesr[:], 1.0)

            # ---- PE warmup: ramp the tensor engine to full clock while
            # the input DMAs stream (identm has no DMA dependency). ----
            with tc.tile_pool(name="pwu", bufs=1, space="PSUM") as PWU:
                wps = PWU.tile([128, 128], f32, tag="wps", name="wps")
                for _ in range(24):
                    nc.tensor.matmul(wps[:], identm[:], identm[:],
                                     start=True, stop=True)

            with tc.tile_pool(name="pa2", bufs=1) as PA2:
                acT = PA2.tile([128, 4, 2, N], f8, tag="acT", name="acT")
                ybp = PA2.tile([128, 4, 2, D], f8, tag="ybp", name="ybp")

                with tc.tile_pool(name="pa1", bufs=1) as PA1:
                    yUT = PA1.tile([128, ND, N], bf, tag="yUT", name="yUT")
                    ts0 = PA1.tile([128, ND, N], bf, tag="ts0", name="ts0")
                    ts1 = PA1.tile([128, ND, N], bf, tag="ts1", name="ts1")

                    # ---------- phase 1: tT, yUT ----------
                    with tc.tile_pool(name="p1", bufs=1) as P1, \
                         tc.tile_pool(name="ps1", bufs=2, space="PSUM") as PS1:
                        utb = P1.tile([128, ND, ND, 128], bf, tag="utb", name="utb")
                        yt = P1.tile([128, ND, N], bf, tag="yt", name="yt")
                        # phase-1-critical inputs first on their queues
                        for d in range(ND):
                            nc.scalar.dma_start(utb[:, d], utb_d[:, d])
                        for k in range(ND):
                            nc.sync.dma_start(xtb[:, k], xtb_d[:, k])
                        for k in range(ND):
                            nc.gpsimd.dma_start(yt[:, k], yt_d[:, k])
                        # first two posn blocks up front; rest stream
                        # just-in-time inside the phase-2 loop
                        for nb in range(2):
                            nc.sync.dma_start(posn[:, nb], posn_d[:, nb])

                        for d in range(ND):
                            ps = PS1.tile([128, N], f32, tag="psA", name="psA")
                            for k in range(ND):
                                for h in range(2):
                                    nc.tensor.matmul(
                                        ps[:, 512 * h : 512 * h + 512],
                                        utb[:, d, k],
                                        xtb[:, k, 512 * h : 512 * h + 512],
                                        start=(k == 0), stop=(k == ND - 1),
                                    )
                            nc.vector.tensor_scalar_mul(
                                ts0[:, d, :], ps[:], s12[:, 0, d : d + 1])
                            nc.scalar.mul(ts1[:, d, :], ps[:], s12[:, 1, d : d + 1])
                        for d in range(ND):
                            ps = PS1.tile([128, N], f32, tag="psA", name="psA")
                            for k in range(ND):
                                for h in range(2):
                                    nc.tensor.matmul(
                                        ps[:, 512 * h : 512 * h + 512],
                                        utb[:, d, k],
                                        yt[:, k, 512 * h : 512 * h + 512],
                                        start=(k == 0), stop=(k == ND - 1),
                                    )
                            nc.scalar.copy(yUT[:, d, :], ps[:])

                    # ---- phase 2: attention, entropy, routing ----
                    # Every engine queue is in-order, so the per-block chain
                    # exp -> combine -> ln -> mult -> reduce -> route -> dka
                    # -> transpose is software-pipelined three deep: loop
                    # step i emits S0(i), S1(i-1), S2(i-2). Each engine then
                    # always has ready work at its queue head.
                    with tc.tile_pool(name="pat", bufs=4) as PT, \
                         tc.tile_pool(name="plk", bufs=3) as LK, \
                         tc.tile_pool(name="pdk", bufs=4) as DK, \
                         tc.tile_pool(name="psm", bufs=8) as SM, \
                         tc.tile_pool(name="psl", bufs=3, space="PSUM") as PSL, \
                         tc.tile_pool(name="pstp", bufs=2, space="PSUM") as PST:

                        # big phase-3/4 weights stream on the sync queue
                        # during phase 2 (it only carries posn + heat stores)
                        w14_loads = (
                            [lambda i=i: nc.sync.dma_start(ybp[:, i], ybp_d[:, i])
                             for i in range(4)]
                            + [lambda: nc.sync.dma_start(b1t[:], b1t_d[:])]
                            + [lambda i=i: nc.sync.dma_start(w1p[:, i], w1p_d[:, i])
                               for i in range(3)]
                            + [lambda i=i: nc.sync.dma_start(w1x[:, i], w1x_d[:, i])
                               for i in range(ND)]
                        )

                        st_ = {}

                        def s0(nb):
                            # logits GEMMs, exp with fused row-sum, 1/esum
                            if 2 + nb < NB:
                                nc.sync.dma_start(posn[:, 2 + nb],
                                                  posn_d[:, 2 + nb])
                            for li in range(2 * nb, min(2 * nb + 2,
                                                        len(w14_loads))):
                                w14_loads[li]()
                            pk = PT.tile([128, 2, N], bf, tag="pk", name="pk")
                            rks = []
                            for k2 in range(2):
                                tsk = ts0 if k2 == 0 else ts1
                                psl = PSL.tile([128, N], f32, tag="psl",
                                               name="psl")
                                for e in range(ND):
                                    for h in range(2):
                                        nc.tensor.matmul(
                                            psl[:, 512 * h : 512 * h + 512],
                                            tsk[:, e, 128 * nb : 128 * nb + 128],
                                            yUT[:, e, 512 * h : 512 * h + 512],
                                            start=(e == 0), stop=(e == ND - 1),
                                        )
                                esum = SM.tile([128, 1], f32, tag="esum",
                                               name="esum")
                                nc.scalar.activation(pk[:, k2, :], psl[:],
                                                     AF.Exp, bias=zerop[:],
                                                     scale=SCALE,
                                                     accum_out=esum[:])
                                rkf = SM.tile([128, 1], f32, tag="rkf",
                                              name="rkf")
                                nc.vector.reciprocal(rkf[:], esum[:])
                                rk = SM.tile([128, 1], bf, tag="rk", name="rk")
                                nc.vector.tensor_scalar_mul(rk[:], rkf[:],
                                                            SA * omg)
                                rks.append(rk)
                            st_[("pk", nb)] = pk
                            st_[("rk", nb)] = rks

                        def s1(nb):
                            # pos combine, ln, entropy product
                            pk = st_[("pk", nb)]
                            rks = st_.pop(("rk", nb))
                            lnk = LK.tile([128, 2, N], bf, tag="lnk",
                                          name="lnk")
                            for k2 in range(2):
                                patch = pk[:, k2, :]
                                nc.vector.scalar_tensor_tensor(
                                    patch, patch, rks[k2][:], posn[:, nb, :],
                                    ALU.mult, ALU.add)
                                nc.scalar.activation(lnk[:, k2, :], patch,
                                                     AF.Ln, bias=epsb[:])
                                # product on DVE (all-bf16 packs 2x and
                                # avoids the shared DVE/Pool SBUF port)
                                nc.vector.tensor_mul(lnk[:, k2, :],
                                                     lnk[:, k2, :], patch)
                            d01 = DK.tile([128, N], bf, tag="d01", name="d01")
                            nc.gpsimd.tensor_sub(d01[:], pk[:, 0, :],
                                                 pk[:, 1, :])
                            st_[("lnk", nb)] = lnk
                            st_[("d01", nb)] = d01

                        def s2(nb):
                            # entropy reduce, routing, heat, dka, transposes
                            r0 = 128 * nb
                            pk = st_.pop(("pk", nb))
                            lnk = st_.pop(("lnk", nb))
                            d01 = st_.pop(("d01", nb))
                            accr = SM.tile([128, 2], f32, tag="accr",
                                           name="accr")
                            # entropy row-sums: k0 on DVE, k1 on ACT via
                            # Copy-with-accumulate (Copy is in every table)
                            nc.vector.tensor_reduce(
                                accr[:, 0:1], lnk[:, 0, :],
                                axis=mybir.AxisListType.X, op=ALU.add)
                            nc.scalar.activation(lnk[:, 1, :], lnk[:, 1, :],
                                                 AF.Copy, bias=0.0,
                                                 accum_out=accr[:, 1:2])
                            # route0 iff H0<=H1 iff accr0>=accr1
                            rsel = SM.tile([128, 1], bf, tag="rsel",
                                           name="rsel")
                            nc.vector.tensor_tensor(rsel[:], accr[:, 0:1],
                                                    accr[:, 1:2], ALU.is_ge)
                            amax = SM.tile([128, 1], f32, tag="amax",
                                           name="amax")
                            nc.vector.tensor_tensor(amax[:], accr[:, 0:1],
                                                    accr[:, 1:2], ALU.max)
                            # e = exp(-ht*H_sel) = exp(ht/256*amax - ht*ln256)
                            ee = SM.tile([128, 1], f32, tag="ee", name="ee")
                            nc.scalar.activation(ee[:], amax[:], AF.Exp,
                                                 scale=ht / SA, bias=hbias[:])
                            ep1 = SM.tile([128, 1], f32, tag="ep1", name="ep1")
                            nc.vector.tensor_scalar_add(ep1[:], ee[:], 1.0)
                            rcp = SM.tile([128, 1], f32, tag="rcp", name="rcp")
                            nc.vector.reciprocal(rcp[:], ep1[:])
                            # heat pre-scaled by 1/SW2 (W2 epilogue reads it
                            # as the only scale on the pso accumulator)
                            heat = SM.tile([128, 1], f32, tag="heat",
                                           name="heat")
                            nc.vector.scalar_tensor_tensor(
                                heat[:], ee[:], 2.0 / SW2, rcp[:],
                                ALU.mult, ALU.mult)
                            nc.sync.dma_start(hmbuf[r0 : r0 + 128, 0:1],
                                              heat[:])
                            dka = DK.tile([128, N], f32t, tag="dka",
                                          name="dka")
                            nc.vector.scalar_tensor_tensor(
                                dka[:], d01[:], rsel[:], pk[:, 1, :],
                                ALU.mult, ALU.add)
                            # 8 transposes into two 512-wide PSUM tiles, one
                            # batched fp8 copy per group of 4
                            for grp in range(2):
                                pst = PST.tile([128, 512], f32t, tag="pst",
                                               name="pst")
                                for q in range(4):
                                    mb = 4 * grp + q
                                    nc.tensor.transpose(
                                        pst[:, 128 * q : 128 * q + 128],
                                        dka[:, 128 * mb : 128 * mb + 128],
                                        identf[:])
                                dst = acT[:, 2 * grp : 2 * grp + 2, :,
                                          r0 : r0 + 128]
                                nc.scalar.copy(dst, pst[:])

                        for i in range(NB + 2):
                            if i < NB:
                                s0(i)
                            if 1 <= i <= NB:
                                s1(i - 1)
                            if i >= 2:
                                s2(i - 2)

                # ---------- phase 3: y_outT (fp8 DoubleRow) -> yf8 ----------
                with tc.tile_pool(name="psy", bufs=2, space="PSUM") as PSY:
                    for d in range(ND):
                        psy = PSY.tile([128, N], f32, tag="psy", name="psy")
                        for mbp in range(4):
                            for h in range(2):
                                mm_dr(
                                    psy[:, 512 * h : 512 * h + 512],
                                    ybp[:, mbp, :, 128 * d : 128 * d + 128],
                                    acT[:, mbp, :, 512 * h : 512 * h + 512],
                                    (mbp == 0), (mbp == 3),
                                )
                        if d % 2 == 0:
                            nc.scalar.mul(yf8[:, d // 2, d % 2, :],
                                          psy[:], SY / SA)
                        else:
                            nc.vector.tensor_scalar_mul(
                                yf8[:, d // 2, d % 2, :], psy[:], SY / SA)

            # ---------- phase 4: MLP ----------
            with tc.tile_pool(name="pg", bufs=1) as PG:
                w2r = PG.tile([128, 12, 2, D], f8, tag="w2r", name="w2r")
                gel = PG.tile([128, 12, 2, N], f8, tag="gel", name="gel")
                nc.sync.dma_start(b2r[:], b2r_d[:])
                for q in range(4):
                    nc.sync.dma_start(w2r[:, 3 * q : 3 * q + 3],
                                      w2r_d[:, 3 * q : 3 * q + 3])

                chunksA = [(0, 512), (512, 512)]
                with tc.tile_pool(name="psh", bufs=2, space="PSUM") as PSH:
                    for f in range(NF):
                        psh = PSH.tile([128, N], f32, tag="psh", name="psh")
                        for c in range(ND):
                            for (s0, wd) in chunksA:
                                nc.tensor.matmul(
                                    psh[:, s0 : s0 + wd],
                                    w1x[:, c, f],
                                    xtb[:, c, s0 : s0 + wd],
                                    start=(c == 0), stop=False,
                                )
                        for yp in range(3):
                            for (s0, wd) in chunksA:
                                mm_dr(
                                    psh[:, s0 : s0 + wd],
                                    w1p[:, yp, f],
                                    yf8[:, yp, :, s0 : s0 + wd],
                                    False, (yp == 2),
                                )
                        nc.scalar.activation(gel[:, f // 2, f % 2, :], psh[:],
                                             AF.Gelu, bias=b1t[:, f : f + 1],
                                             scale=1.0 / SW1X)

                with tc.tile_pool(name="p5", bufs=3) as P5, \
                     tc.tile_pool(name="pso", bufs=2, space="PSUM") as PSO:
                    chunksB = [(512, D - 512), (0, 512)]
                    for tb in range(NB):
                        r0 = 128 * tb
                        xn = P5.tile([128, D], f32, tag="xn", name="xn")
                        nc.sync.dma_start(xn[:], xnat_d[r0 : r0 + 128, :])
                        hmc = P5.tile([128, 1], f32, tag="hmc", name="hmc")
                        nc.sync.dma_start(hmc[:], hmbuf[r0 : r0 + 128, 0:1])
                        pso = PSO.tile([128, D], f32, tag="pso", name="pso")
                        for fp in range(12):
                            for (s0, wd) in chunksB:
                                mm_dr(
                                    pso[:, s0 : s0 + wd],
                                    gel[:, fp, :, r0 : r0 + 128],
                                    w2r[:, fp, :, s0 : s0 + wd],
                                    (fp == 0), False,
                                )
                        # rank-1 b2 bias fold: pso += ones.T @ (SW2*b2)
                        for (s0, wd) in chunksB:
                            nc.tensor.matmul(
                                pso[:, s0 : s0 + wd],
                                onesr[0:1, 0:128],
                                b2r[0:1, s0 : s0 + wd],
                                start=False, stop=True,
                            )
                        # single drain op: out = pso*(heat/SW2) + x
                        ot = P5.tile([128, D], f32, tag="ot", name="ot")
                        nc.vector.scalar_tensor_tensor(
                            ot[:], pso[:], hmc[:], xn[:],
                            ALU.mult, ALU.add)
                        nc.sync.dma_start(out_d[r0 : r0 + 128, :], ot[:])

    nc.compile()
    return nc


def _get_prog(g, ht, pt):
    key = (round(float(g), 9), round(float(ht), 9), round(float(pt), 9))
    if key not in _prog_cache:
        _prog_cache[key] = _build(*key)
    return _prog_cache[key]


def kernel(x, y, coords, U, S1, S2, gating, h_temp, p_temp, pos_emb, W1, b1, W2, b2):
    x = np.asarray(x, dtype=np.float32)
    y = np.asarray(y, dtype=np.float32)
    coords = np.asarray(coords, dtype=np.float32)
    U = np.asarray(U, dtype=np.float32)
    bf16 = ml_dtypes.bfloat16
    f8 = ml_dtypes.float8_e4m3

    g = float(1.0 / (1.0 + np.exp(-float(np.asarray(gating)))))
    ht = float(np.asarray(h_temp))
    pt = abs(float(np.asarray(p_temp)))
    nc = _get_prog(g, ht, pt)

    def q8(a):
        return np.clip(a, -240.0, 240.0).astype(f8)

    # ---- shared (replicated) host prep ----
    UT = np.ascontiguousarray(U.T)
    utb = np.ascontiguousarray(
        UT.reshape(ND, 128, ND, 128).transpose(1, 2, 0, 3)).astype(bf16)
    s12 = np.ascontiguousarray(np.stack(
        [np.abs(np.asarray(S1, np.float32)).reshape(ND, 128).T,
         np.abs(np.asarray(S2, np.float32)).reshape(ND, 128).T], axis=1))
    # pos softmax on host (exact):
    #   posn[i, j] = SA*g*softmax_j(-pt * sum_c coords[i,j,c]*pe[i,c])
    pe_f = np.asarray(pos_emb, np.float32)[:, :, 0]            # [N, 6]
    pl = -pt * np.einsum("ijc,ic->ij", coords, pe_f)           # [N, N]
    pl -= pl.max(axis=1, keepdims=True)
    pexp = np.exp(pl)
    psm = pexp / pexp.sum(axis=1, keepdims=True)
    posn = np.ascontiguousarray(
        (SA * g * psm).reshape(NB, 128, N).transpose(1, 0, 2)).astype(bf16)
    W1 = np.asarray(W1, np.float32)
    W1a, W1b = W1[:D], W1[D:]
    w1x = np.ascontiguousarray(
        (SW1X * W1a).reshape(ND, 128, NF, 128).transpose(1, 0, 2, 3)).astype(bf16)
    w1p = q8((SW1Y * W1b).reshape(3, 2, 128, NF, 128).transpose(2, 0, 3, 1, 4))
    W2 = np.asarray(W2, np.float32)
    w2r = q8((SW2 * W2).reshape(12, 2, 128, D).transpose(2, 0, 1, 3))
    b1 = np.asarray(b1, np.float32)
    b2 = np.asarray(b2, np.float32)
    b1t = np.ascontiguousarray(b1.reshape(NF, 128).T)
    b2r = np.ascontiguousarray((SW2 * b2).reshape(1, D)).astype(bf16)

    shared = {"utb": utb, "s12": s12, "posn": posn,
              "w1x": w1x, "w1p": w1p, "w2r": w2r, "b1t": b1t, "b2r": b2r}

    in_maps = []
    cls_rows = []
    for b in range(B):
        xb = x[b, 1:]
        yb = y[b, 1:]
        xtb = np.ascontiguousarray(
            xb.T.reshape(ND, 128, N).transpose(1, 0, 2)).astype(bf16)
        yt = np.ascontiguousarray(
            yb.T.reshape(ND, 128, N).transpose(1, 0, 2)).astype(bf16)
        ybp = q8(yb.reshape(4, 2, 128, D).transpose(2, 0, 1, 3))
        # CLS output row on host (exact f32 1x1536 MLP; no heat scaling)
        h0 = np.concatenate([x[b, 0], y[b, 0]]) @ W1 + b1
        xp0 = (h0 * 0.5 * (1.0 + erf(h0 / np.sqrt(2.0)))) @ W2 + b2
        cls_rows.append(x[b, 0] + xp0)
        m = dict(shared)
        m["xtb"] = xtb
        m["yt"] = yt
        m["ybp"] = ybp
        m["xnat"] = np.ascontiguousarray(xb)
        in_maps.append(m)

    res = run_bass_kernel_spmd(nc, in_maps, list(range(B)))
    out = np.empty((B, N + 1, D), np.float32)
    for b in range(B):
        out[b, 0] = cls_rows[b]
        out[b, 1:] = res.results[b]["out"]
    return out


if __name__ == "__main__":
    import time
    sys.path.insert(0, "/root/problem")
    from reference import setup_inputs, reference

    inp = {k: np.asarray(v) for k, v in setup_inputs().items()}
    t0 = time.time()
    got = kernel(**inp)
    print("kernel wall:", time.time() - t0)
    exp = np.asarray(reference(**inp))
    d = np.abs(got - exp)
    print("absmax_rel:", d.max() / np.abs(exp).max())
    print("rms_rel:", np.sqrt((d ** 2).mean()) / np.sqrt((exp ** 2).mean()))


# revision 55
# speedup vs baseline: 1.2143x; 1.1265x over previous
"""Trainium2 Bass kernel for nn_DepthCueRectification_Sp.

Data-parallel over batch: 8 batch elements -> 8 NeuronCores (SPMD).

Host precomputes (exact, f32): pos softmax (posn = SA*g*softmax(pos_logits)),
the CLS output row (tiny 1x1536 MLP), and all weight/layout prep. The device
handles only the N=1024 patch tokens - no padding, no CLS special cases.

Per-core pipeline (D=768, N=1024):
  tT    = U @ xb.T                  (bf16)
  yUT   = U @ yb.T                  (bf16)   [algebra: logits_k =
                                     (|S_k|*t) @ (y@U.T).T]
  tsT_k = |S_k|-scaled copies of tT (ACT per-partition scale)
  logits_k -> exp (no max-sub, fused row-sum) -> attn_k = 256*attn (bf16)
  patch_k += posn (host-exact pos term, bf16)
  entropy: Ln on ACT; fused mult+row-sum via DVE scalar_tensor_tensor
  accum_out; routing compares raw accums; heat = 2e/(1+e) via one exp
  dka (selected attn, f32r) -> PE transpose -> acT fp8 pairs
  y_outT = fp8 DoubleRow (ybp pairs @ acT pairs) -> yf8 = 16*y_out.T fp8
  MLP W1: x-half bf16 (xtb @ 32*W1a) + y-half fp8 DR (yf8 @ 2*W1b) = 32*h
  gel   = gelu(psh/32 + b1) -> fp8 pairs
  MLP W2: fp8 DoubleRow (gel pairs @ 32*W2 pairs) = 32*xp'
  out   = x + heat*(xp' + b2)

The act-table dict is patched so Exp and Ln resolve to the combined
natural_log_exp_and_others set (avoids per-iteration table reloads).
tensor_tensor_reduce and 16-bit PE transposes hard-crash the exec unit
on this toolchain and are not used.
"""

import os
import sys

if "/opt/trn_rl_repo" not in sys.path:
    sys.path.insert(0, "/opt/trn_rl_repo")

import numpy as np
import ml_dtypes
from scipy.special import erf

import concourse.bass as bass
import concourse.bass_utils as _bu
import concourse.mybir as mybir
import concourse.tile as tile
from concourse import bacc
from concourse.bass_utils import run_bass_kernel_spmd
from concourse.hw_specs import get_activation_tables
from concourse.masks import make_identity

B, N, D, DFF = 8, 1024, 768, 3072
ND = D // 128          # 6
NB = N // 128          # 8
NF = DFF // 128        # 24
AF = mybir.ActivationFunctionType
ALU = mybir.AluOpType
dt = mybir.dt
DR = mybir.MatmulPerfMode.DoubleRow

NODR = bool(int(os.environ.get("K_NODR", "0")))      # disable DoubleRow
TR32R = bool(int(os.environ.get("K_TR32R", "0")))    # f32r PE transposes

SCALE = float(D) ** -0.5
SA = 256.0             # attn scale (fp8 headroom)
SY = 16.0              # y_full scale in yf8
SW1X = 32.0            # W1 x-half scale (bf16)  == SW1Y*SY
SW1Y = 2.0             # W1 y-half scale (fp8)
SW2 = 32.0             # W2 scale (fp8)
LN256 = float(np.log(SA))

_prog_cache = {}


def _patch_act_tables(arch):
    """Make natural_log_exp_and_others the only provider of Exp/Ln so the
    compiler's table-load pass keeps one table across the attention loop.
    Mutates the functools-cached dict in place (names/ids unchanged)."""
    tabs = get_activation_tables(arch)
    keep = "natural_log_exp_and_others"
    if keep not in tabs:
        return
    for name, s in tabs.items():
        if name == keep:
            continue
        s.discard(AF.Exp)
        s.discard(AF.Ln)


def _build(g, ht, pt):
    omg = 1.0 - g
    f8 = dt.float8e4
    bf = dt.bfloat16
    f32 = dt.float32
    f32t = dt.float32r if TR32R else dt.float32

    nc = bacc.Bacc("TRN2", target_bir_lowering=False, debug=False, num_devices=8)
    _patch_act_tables(nc.m.arch)

    def mm_dr(out, l3, r3, start, stop):
        if not NODR:
            nc.tensor.matmul(out, l3, r3, start=start, stop=stop, perf_mode=DR)
        else:
            nc.tensor.matmul(out, l3[:, 0], r3[:, 0], start=start, stop=False)
            nc.tensor.matmul(out, l3[:, 1], r3[:, 1], start=False, stop=stop)

    # ---- DRAM params ----
    xtb_d = nc.declare_dram_parameter("xtb", [128, ND, N], bf, isOutput=False)
    yt_d = nc.declare_dram_parameter("yt", [128, ND, N], bf, isOutput=False)
    ybp_d = nc.declare_dram_parameter("ybp", [128, 4, 2, D], f8, isOutput=False)
    utb_d = nc.declare_dram_parameter("utb", [128, ND, ND, 128], bf, isOutput=False)
    posn_d = nc.declare_dram_parameter("posn", [128, NB, N], bf, isOutput=False)
    w1p_d = nc.declare_dram_parameter("w1p", [128, 3, NF, 2, 128], f8, isOutput=False)
    w1x_d = nc.declare_dram_parameter("w1x", [128, 3, NF, 2, 128], f8, isOutput=False)
    xf8_d = nc.declare_dram_parameter("xf8", [128, 3, 2, N], f8, isOutput=False)
    w2r_d = nc.declare_dram_parameter("w2r", [128, 12, 2, D], f8, isOutput=False)
    b1t_d = nc.declare_dram_parameter("b1t", [128, NF], f32, isOutput=False)
    b2r_d = nc.declare_dram_parameter("b2r", [1, D], bf, isOutput=False)
    s12_d = nc.declare_dram_parameter("s12", [128, 2, ND], f32, isOutput=False)
    xnat_d = nc.declare_dram_parameter("xnat", [N, D], f32, isOutput=False)
    out_d = nc.declare_dram_parameter("out", [N, D], f32, isOutput=True)
    hmbuf = nc.dram_tensor("hmbuf", [N, 1], f32)

    with tile.TileContext(nc) as tc:
        with tc.tile_pool(name="p0", bufs=1) as P0:
            # ---- persistent tiles ----
            w1p = P0.tile([128, 3, NF, 2, 128], f8, tag="w1p", name="w1p")
            w1x = P0.tile([128, 3, NF, 2, 128], f8, tag="w1x", name="w1x")
            xf8 = P0.tile([128, 3, 2, N], f8, tag="xf8", name="xf8")
            xtb = P0.tile([128, ND, N], bf, tag="xtb", name="xtb")
            yf8 = P0.tile([128, 3, 2, N], f8, tag="yf8", name="yf8")
            posn = P0.tile([128, NB, N], bf, tag="posn", name="posn")
            b2r = P0.tile([1, D], bf, tag="b2r", name="b2r")
            b1t = P0.tile([128, NF], f32, tag="b1t", name="b1t")
            s12 = P0.tile([128, 2, ND], f32, tag="s12", name="s12")
            identf = P0.tile([128, 128], f32, tag="identf", name="identf")
            identm = P0.tile([128, 128], bf, tag="identm", name="identm")
            onesr = P0.tile([1, 128], bf, tag="onesr", name="onesr")
            epsb = P0.tile([128, 1], f32, tag="epsb", name="epsb")
            zerop = P0.tile([128, 1], f32, tag="zerop", name="zerop")
            hbias = P0.tile([128, 1], f32, tag="hbias", name="hbias")

            # ---- gpsimd queue: s12 (phase-1 tail dep) + small inits ----
            nc.gpsimd.dma_start(s12[:], s12_d[:])
            make_identity(nc, identf[:])
            make_identity(nc, identm[:])
            nc.gpsimd.memset(epsb[:], SA * 1e-8)
            nc.gpsimd.memset(hbias[:], -ht * LN256)
            nc.gpsimd.memset(zerop[:], 0.0)
            nc.gpsimd.memset(onesr[:], 1.0)

            # ---- PE warmup: ramp the tensor engine to full clock while
            # the input DMAs stream (identm has no DMA dependency). ----
            with tc.tile_pool(name="pwu", bufs=1, space="PSUM") as PWU:
                wps = PWU.tile([128, 128], f32, tag="wps", name="wps")
                for _ in range(24):
                    nc.tensor.matmul(wps[:], identm[:], identm[:],
                                     start=True, stop=True)

            with tc.tile_pool(name="pa2", bufs=1) as PA2:
                acT = PA2.tile([128, 4, 2, N], f8, tag="acT", name="acT")
                ybp = PA2.tile([128, 4, 2, D], f8, tag="ybp", name="ybp")

                with tc.tile_pool(name="pa1", bufs=1) as PA1:
                    yUT = PA1.tile([128, ND, N], bf, tag="yUT", name="yUT")
                    ts0 = PA1.tile([128, ND, N], bf, tag="ts0", name="ts0")
                    ts1 = PA1.tile([128, ND, N], bf, tag="ts1", name="ts1")

                    # ---------- phase 1: tT, yUT ----------
                    with tc.tile_pool(name="p1", bufs=1) as P1, \
                         tc.tile_pool(name="ps1", bufs=2, space="PSUM") as PS1:
                        utb = P1.tile([128, ND, ND, 128], bf, tag="utb", name="utb")
                        yt = P1.tile([128, ND, N], bf, tag="yt", name="yt")
                        # phase-1-critical inputs first on their queues
                        for d in range(ND):
                            nc.scalar.dma_start(utb[:, d], utb_d[:, d])
                        for k in range(ND):
                            nc.sync.dma_start(xtb[:, k], xtb_d[:, k])
                        for k in range(ND):
                            nc.gpsimd.dma_start(yt[:, k], yt_d[:, k])
                        # first two posn blocks up front; rest stream
                        # just-in-time inside the phase-2 loop
                        for nb in range(2):
                            nc.sync.dma_start(posn[:, nb], posn_d[:, nb])

                        for d in range(ND):
                            ps = PS1.tile([128, N], f32, tag="psA", name="psA")
                            for k in range(ND):
                                for h in range(2):
                                    nc.tensor.matmul(
                                        ps[:, 512 * h : 512 * h + 512],
                                        utb[:, d, k],
                                        xtb[:, k, 512 * h : 512 * h + 512],
                                        start=(k == 0), stop=(k == ND - 1),
                                    )
                            nc.vector.tensor_scalar_mul(
                                ts0[:, d, :], ps[:], s12[:, 0, d : d + 1])
                            nc.scalar.mul(ts1[:, d, :], ps[:], s12[:, 1, d : d + 1])
                        for d in range(ND):
                            ps = PS1.tile([128, N], f32, tag="psA", name="psA")
                            for k in range(ND):
                                for h in range(2):
                                    nc.tensor.matmul(
                                        ps[:, 512 * h : 512 * h + 512],
                                        utb[:, d, k],
                                        yt[:, k, 512 * h : 512 * h + 512],
                                        start=(k == 0), stop=(k == ND - 1),
                                    )
                            nc.scalar.copy(yUT[:, d, :], ps[:])

                    # ---- phase 2: attention, entropy, routing ----
                    # Every engine queue is in-order, so the per-block chain
                    # exp -> combine -> ln -> mult -> reduce -> route -> dka
                    # -> transpose is software-pipelined three deep: loop
                    # step i emits S0(i), S1(i-1), S2(i-2). Each engine then
                    # always has ready work at its queue head.
                    with tc.tile_pool(name="pat", bufs=4) as PT, \
                         tc.tile_pool(name="plk", bufs=3) as LK, \
                         tc.tile_pool(name="pdk", bufs=4) as DK, \
                         tc.tile_pool(name="psm", bufs=8) as SM, \
                         tc.tile_pool(name="psl", bufs=3, space="PSUM") as PSL, \
                         tc.tile_pool(name="pstp", bufs=2, space="PSUM") as PST:

                        # big phase-3/4 weights stream on the sync queue
                        # during phase 2 (it only carries posn + heat stores)
                        w14_loads = (
                            [lambda i=i: nc.sync.dma_start(ybp[:, i], ybp_d[:, i])
                             for i in range(4)]
                            + [lambda: nc.sync.dma_start(b1t[:], b1t_d[:])]
                            + [lambda: nc.sync.dma_start(xf8[:], xf8_d[:])]
                            + [lambda i=i: nc.sync.dma_start(w1p[:, i], w1p_d[:, i])
                               for i in range(3)]
                            + [lambda i=i: nc.sync.dma_start(w1x[:, i], w1x_d[:, i])
                               for i in range(3)]
                        )

                        st_ = {}

                        def s0(nb):
                            # logits GEMMs, exp with fused row-sum, 1/esum
                            if 2 + nb < NB:
                                nc.sync.dma_start(posn[:, 2 + nb],
                                                  posn_d[:, 2 + nb])
                            for li in range(2 * nb, min(2 * nb + 2,
                                                        len(w14_loads))):
                                w14_loads[li]()
                            pk = PT.tile([128, 2, N], bf, tag="pk", name="pk")
                            rks = []
                            for k2 in range(2):
                                tsk = ts0 if k2 == 0 else ts1
                                psl = PSL.tile([128, N], f32, tag="psl",
                                               name="psl")
                                for e in range(ND):
                                    for h in range(2):
                                        nc.tensor.matmul(
                                            psl[:, 512 * h : 512 * h + 512],
                                            tsk[:, e, 128 * nb : 128 * nb + 128],
                                            yUT[:, e, 512 * h : 512 * h + 512],
                                            start=(e == 0), stop=(e == ND - 1),
                                        )
                                esum = SM.tile([128, 1], f32, tag="esum",
                                               name="esum")
                                nc.scalar.activation(pk[:, k2, :], psl[:],
                                                     AF.Exp, bias=zerop[:],
                                                     scale=SCALE,
                                                     accum_out=esum[:])
                                rkf = SM.tile([128, 1], f32, tag="rkf",
                                              name="rkf")
                                nc.vector.reciprocal(rkf[:], esum[:])
                                rk = SM.tile([128, 1], bf, tag="rk", name="rk")
                                nc.vector.tensor_scalar_mul(rk[:], rkf[:],
                                                            SA * omg)
                                rks.append(rk)
                            st_[("pk", nb)] = pk
                            st_[("rk", nb)] = rks

                        def s1(nb):
                            # pos combine, ln, entropy product
                            pk = st_[("pk", nb)]
                            rks = st_.pop(("rk", nb))
                            lnk = LK.tile([128, 2, N], bf, tag="lnk",
                                          name="lnk")
                            for k2 in range(2):
                                patch = pk[:, k2, :]
                                nc.vector.scalar_tensor_tensor(
                                    patch, patch, rks[k2][:], posn[:, nb, :],
                                    ALU.mult, ALU.add)
                                nc.scalar.activation(lnk[:, k2, :], patch,
                                                     AF.Ln, bias=epsb[:])
                                # product on DVE (all-bf16 packs 2x and
                                # avoids the shared DVE/Pool SBUF port)
                                nc.vector.tensor_mul(lnk[:, k2, :],
                                                     lnk[:, k2, :], patch)
                            d01 = DK.tile([128, N], bf, tag="d01", name="d01")
                            nc.gpsimd.tensor_sub(d01[:], pk[:, 0, :],
                                                 pk[:, 1, :])
                            st_[("lnk", nb)] = lnk
                            st_[("d01", nb)] = d01

                        def s2(nb):
                            # entropy reduce, routing, heat, dka, transposes
                            r0 = 128 * nb
                            pk = st_.pop(("pk", nb))
                            lnk = st_.pop(("lnk", nb))
                            d01 = st_.pop(("d01", nb))
                            accr = SM.tile([128, 2], f32, tag="accr",
                                           name="accr")
                            # entropy row-sums: k0 on DVE, k1 on ACT via
                            # Copy-with-accumulate (Copy is in every table)
                            nc.vector.tensor_reduce(
                                accr[:, 0:1], lnk[:, 0, :],
                                axis=mybir.AxisListType.X, op=ALU.add)
                            nc.scalar.activation(lnk[:, 1, :], lnk[:, 1, :],
                                                 AF.Copy, bias=0.0,
                                                 accum_out=accr[:, 1:2])
                            # route0 iff H0<=H1 iff accr0>=accr1
                            rsel = SM.tile([128, 1], bf, tag="rsel",
                                           name="rsel")
                            nc.vector.tensor_tensor(rsel[:], accr[:, 0:1],
                                                    accr[:, 1:2], ALU.is_ge)
                            amax = SM.tile([128, 1], f32, tag="amax",
                                           name="amax")
                            nc.vector.tensor_tensor(amax[:], accr[:, 0:1],
                                                    accr[:, 1:2], ALU.max)
                            # e = exp(-ht*H_sel) = exp(ht/256*amax - ht*ln256)
                            ee = SM.tile([128, 1], f32, tag="ee", name="ee")
                            nc.scalar.activation(ee[:], amax[:], AF.Exp,
                                                 scale=ht / SA, bias=hbias[:])
                            ep1 = SM.tile([128, 1], f32, tag="ep1", name="ep1")
                            nc.vector.tensor_scalar_add(ep1[:], ee[:], 1.0)
                            rcp = SM.tile([128, 1], f32, tag="rcp", name="rcp")
                            nc.vector.reciprocal(rcp[:], ep1[:])
                            # heat pre-scaled by 1/SW2 (W2 epilogue reads it
                            # as the only scale on the pso accumulator)
                            heat = SM.tile([128, 1], f32, tag="heat",
                                           name="heat")
                            nc.vector.scalar_tensor_tensor(
                                heat[:], ee[:], 2.0 / SW2, rcp[:],
                                ALU.mult, ALU.mult)
                            nc.sync.dma_start(hmbuf[r0 : r0 + 128, 0:1],
                                              heat[:])
                            dka = DK.tile([128, N], f32t, tag="dka",
                                          name="dka")
                            nc.vector.scalar_tensor_tensor(
                                dka[:], d01[:], rsel[:], pk[:, 1, :],
                                ALU.mult, ALU.add)
                            # 8 transposes into two 512-wide PSUM tiles, one
                            # batched fp8 copy per group of 4
                            for grp in range(2):
                                pst = PST.tile([128, 512], f32t, tag="pst",
                                               name="pst")
                                for q in range(4):
                                    mb = 4 * grp + q
                                    nc.tensor.transpose(
                                        pst[:, 128 * q : 128 * q + 128],
                                        dka[:, 128 * mb : 128 * mb + 128],
                                        identf[:])
                                dst = acT[:, 2 * grp : 2 * grp + 2, :,
                                          r0 : r0 + 128]
                                nc.scalar.copy(dst, pst[:])

                        for i in range(NB + 2):
                            if i < NB:
                                s0(i)
                            if 1 <= i <= NB:
                                s1(i - 1)
                            if i >= 2:
                                s2(i - 2)

                # ---------- phase 3: y_outT (fp8 DoubleRow) -> yf8 ----------
                with tc.tile_pool(name="psy", bufs=2, space="PSUM") as PSY:
                    for d in range(ND):
                        psy = PSY.tile([128, N], f32, tag="psy", name="psy")
                        for mbp in range(4):
                            for h in range(2):
                                mm_dr(
                                    psy[:, 512 * h : 512 * h + 512],
                                    ybp[:, mbp, :, 128 * d : 128 * d + 128],
                                    acT[:, mbp, :, 512 * h : 512 * h + 512],
                                    (mbp == 0), (mbp == 3),
                                )
                        if d % 2 == 0:
                            nc.scalar.mul(yf8[:, d // 2, d % 2, :],
                                          psy[:], SY / SA)
                        else:
                            nc.vector.tensor_scalar_mul(
                                yf8[:, d // 2, d % 2, :], psy[:], SY / SA)

            # ---------- phase 4: MLP ----------
            with tc.tile_pool(name="pg", bufs=1) as PG:
                w2r = PG.tile([128, 12, 2, D], f8, tag="w2r", name="w2r")
                gel = PG.tile([128, 12, 2, N], f8, tag="gel", name="gel")
                nc.sync.dma_start(b2r[:], b2r_d[:])
                for q in range(4):
                    nc.sync.dma_start(w2r[:, 3 * q : 3 * q + 3],
                                      w2r_d[:, 3 * q : 3 * q + 3])

                chunksA = [(0, 512), (512, 512)]
                with tc.tile_pool(name="psh", bufs=2, space="PSUM") as PSH:
                    for f in range(NF):
                        psh = PSH.tile([128, N], f32, tag="psh", name="psh")
                        for xp in range(3):
                            for (s0, wd) in chunksA:
                                mm_dr(
                                    psh[:, s0 : s0 + wd],
                                    w1x[:, xp, f],
                                    xf8[:, xp, :, s0 : s0 + wd],
                                    (xp == 0), False,
                                )
                        for yp in range(3):
                            for (s0, wd) in chunksA:
                                mm_dr(
                                    psh[:, s0 : s0 + wd],
                                    w1p[:, yp, f],
                                    yf8[:, yp, :, s0 : s0 + wd],
                                    False, (yp == 2),
                                )
                        nc.scalar.activation(gel[:, f // 2, f % 2, :], psh[:],
                                             AF.Gelu, bias=b1t[:, f : f + 1],
                                             scale=1.0 / SW1X)

                with tc.tile_pool(name="p5", bufs=3) as P5, \
                     tc.tile_pool(name="pso", bufs=2, space="PSUM") as PSO:
                    chunksB = [(512, D - 512), (0, 512)]
                    for tb in range(NB):
                        r0 = 128 * tb
                        xn = P5.tile([128, D], f32, tag="xn", name="xn")
                        nc.sync.dma_start(xn[:], xnat_d[r0 : r0 + 128, :])
                        hmc = P5.tile([128, 1], f32, tag="hmc", name="hmc")
                        nc.sync.dma_start(hmc[:], hmbuf[r0 : r0 + 128, 0:1])
                        pso = PSO.tile([128, D], f32, tag="pso", name="pso")
                        for fp in range(12):
                            for (s0, wd) in chunksB:
                                mm_dr(
                                    pso[:, s0 : s0 + wd],
                                    gel[:, fp, :, r0 : r0 + 128],
                                    w2r[:, fp, :, s0 : s0 + wd],
                                    (fp == 0), False,
                                )
                        # rank-1 b2 bias fold: pso += ones.T @ (SW2*b2)
                        for (s0, wd) in chunksB:
                            nc.tensor.matmul(
                                pso[:, s0 : s0 + wd],
                                onesr[0:1, 0:128],
                                b2r[0:1, s0 : s0 + wd],
                                start=False, stop=True,
                            )
                        # single drain op: out = pso*(heat/SW2) + x
                        ot = P5.tile([128, D], f32, tag="ot", name="ot")
                        nc.vector.scalar_tensor_tensor(
                            ot[:], pso[:], hmc[:], xn[:],
                            ALU.mult, ALU.add)
                        nc.sync.dma_start(out_d[r0 : r0 + 128, :], ot[:])

    nc.compile()
    return nc


def _get_prog(g, ht, pt):
    key = (round(float(g), 9), round(float(ht), 9), round(float(pt), 9))
    if key not in _prog_cache:
        _prog_cache[key] = _build(*key)
    return _prog_cache[key]


def kernel(x, y, coords, U, S1, S2, gating, h_temp, p_temp, pos_emb, W1, b1, W2, b2):
    x = np.asarray(x, dtype=np.float32)
    y = np.asarray(y, dtype=np.float32)
    coords = np.asarray(coords, dtype=np.float32)
    U = np.asarray(U, dtype=np.float32)
    bf16 = ml_dtypes.bfloat16
    f8 = ml_dtypes.float8_e4m3

    g = float(1.0 / (1.0 + np.exp(-float(np.asarray(gating)))))
    ht = float(np.asarray(h_temp))
    pt = abs(float(np.asarray(p_temp)))
    nc = _get_prog(g, ht, pt)

    def q8(a):
        return np.clip(a, -240.0, 240.0).astype(f8)

    # ---- shared (replicated) host prep ----
    UT = np.ascontiguousarray(U.T)
    utb = np.ascontiguousarray(
        UT.reshape(ND, 128, ND, 128).transpose(1, 2, 0, 3)).astype(bf16)
    s12 = np.ascontiguousarray(np.stack(
        [np.abs(np.asarray(S1, np.float32)).reshape(ND, 128).T,
         np.abs(np.asarray(S2, np.float32)).reshape(ND, 128).T], axis=1))
    # pos softmax on host (exact):
    #   posn[i, j] = SA*g*softmax_j(-pt * sum_c coords[i,j,c]*pe[i,c])
    pe_f = np.asarray(pos_emb, np.float32)[:, :, 0]            # [N, 6]
    pl = -pt * np.einsum("ijc,ic->ij", coords, pe_f)           # [N, N]
    pl -= pl.max(axis=1, keepdims=True)
    pexp = np.exp(pl)
    psm = pexp / pexp.sum(axis=1, keepdims=True)
    posn = np.ascontiguousarray(
        (SA * g * psm).reshape(NB, 128, N).transpose(1, 0, 2)).astype(bf16)
    W1 = np.asarray(W1, np.float32)
    W1a, W1b = W1[:D], W1[D:]
    w1x = q8((SW1Y * W1a).reshape(3, 2, 128, NF, 128).transpose(2, 0, 3, 1, 4))
    w1p = q8((SW1Y * W1b).reshape(3, 2, 128, NF, 128).transpose(2, 0, 3, 1, 4))
    W2 = np.asarray(W2, np.float32)
    w2r = q8((SW2 * W2).reshape(12, 2, 128, D).transpose(2, 0, 1, 3))
    b1 = np.asarray(b1, np.float32)
    b2 = np.asarray(b2, np.float32)
    b1t = np.ascontiguousarray(b1.reshape(NF, 128).T)
    b2r = np.ascontiguousarray((SW2 * b2).reshape(1, D)).astype(bf16)

    shared = {"utb": utb, "s12": s12, "posn": posn,
              "w1x": w1x, "w1p": w1p, "w2r": w2r, "b1t": b1t, "b2r": b2r}

    in_maps = []
    cls_rows = []
    for b in range(B):
        xb = x[b, 1:]
        yb = y[b, 1:]
        xtb = np.ascontiguousarray(
            xb.T.reshape(ND, 128, N).transpose(1, 0, 2)).astype(bf16)
        yt = np.ascontiguousarray(
            yb.T.reshape(ND, 128, N).transpose(1, 0, 2)).astype(bf16)
        ybp = q8(yb.reshape(4, 2, 128, D).transpose(2, 0, 1, 3))
        xf8 = q8(SY * xb.T.reshape(3, 2, 128, N).transpose(2, 0, 1, 3))
        # CLS output row on host (exact f32 1x1536 MLP; no heat scaling)
        h0 = np.concatenate([x[b, 0], y[b, 0]]) @ W1 + b1
        xp0 = (h0 * 0.5 * (1.0 + erf(h0 / np.sqrt(2.0)))) @ W2 + b2
        cls_rows.append(x[b, 0] + xp0)
        m = dict(shared)
        m["xtb"] = xtb
        m["yt"] = yt
        m["ybp"] = ybp
        m["xf8"] = xf8
        m["xnat"] = np.ascontiguousarray(xb)
        in_maps.append(m)

    res = run_bass_kernel_spmd(nc, in_maps, list(range(B)))
    out = np.empty((B, N + 1, D), np.float32)
    for b in range(B):
        out[b, 0] = cls_rows[b]
        out[b, 1:] = res.results[b]["out"]
    return out


if __name__ == "__main__":
    import time
    sys.path.insert(0, "/root/problem")
    from reference import setup_inputs, reference

    inp = {k: np.asarray(v) for k, v in setup_inputs().items()}
    t0 = time.time()
    got = kernel(**inp)
    print("kernel wall:", time.time() - t0)
    exp = np.asarray(reference(**inp))
    d = np.abs(got - exp)
    print("absmax_rel:", d.max() / np.abs(exp).max())
    print("rms_rel:", np.sqrt((d ** 2).mean()) / np.sqrt((exp ** 2).mean()))


# revision 58
# speedup vs baseline: 1.2336x; 1.0159x over previous
"""Trainium2 Bass kernel for nn_DepthCueRectification_Sp.

Data-parallel over batch: 8 batch elements -> 8 NeuronCores (SPMD).

Host precomputes (exact, f32): pos softmax (posn = SA*g*softmax(pos_logits)),
the CLS output row (tiny 1x1536 MLP), and all weight/layout prep. The device
handles only the N=1024 patch tokens - no padding, no CLS special cases.

Per-core pipeline (D=768, N=1024):
  tT    = U @ xb.T                  (bf16)
  yUT   = U @ yb.T                  (bf16)   [algebra: logits_k =
                                     (|S_k|*t) @ (y@U.T).T]
  tsT_k = |S_k|-scaled copies of tT (ACT per-partition scale)
  logits_k -> exp (no max-sub, fused row-sum) -> attn_k = 256*attn (bf16)
  patch_k += posn (host-exact pos term, bf16)
  entropy: Ln on ACT; fused mult+row-sum via DVE scalar_tensor_tensor
  accum_out; routing compares raw accums; heat = 2e/(1+e) via one exp
  dka (selected attn, f32r) -> PE transpose -> acT fp8 pairs
  y_outT = fp8 DoubleRow (ybp pairs @ acT pairs) -> yf8 = 16*y_out.T fp8
  MLP W1: x-half bf16 (xtb @ 32*W1a) + y-half fp8 DR (yf8 @ 2*W1b) = 32*h
  gel   = gelu(psh/32 + b1) -> fp8 pairs
  MLP W2: fp8 DoubleRow (gel pairs @ 32*W2 pairs) = 32*xp'
  out   = x + heat*(xp' + b2)

The act-table dict is patched so Exp and Ln resolve to the combined
natural_log_exp_and_others set (avoids per-iteration table reloads).
tensor_tensor_reduce and 16-bit PE transposes hard-crash the exec unit
on this toolchain and are not used.
"""

import os
import sys

if "/opt/trn_rl_repo" not in sys.path:
    sys.path.insert(0, "/opt/trn_rl_repo")

import numpy as np
import ml_dtypes
from scipy.special import erf

import concourse.bass as bass
import concourse.bass_utils as _bu
import concourse.mybir as mybir
import concourse.tile as tile
from concourse import bacc
from concourse.bass_utils import run_bass_kernel_spmd
from concourse.hw_specs import get_activation_tables
from concourse.masks import make_identity

B, N, D, DFF = 8, 1024, 768, 3072
ND = D // 128          # 6
NB = N // 128          # 8
NF = DFF // 128        # 24
AF = mybir.ActivationFunctionType
ALU = mybir.AluOpType
dt = mybir.dt
DR = mybir.MatmulPerfMode.DoubleRow

NODR = bool(int(os.environ.get("K_NODR", "0")))      # disable DoubleRow
TR32R = bool(int(os.environ.get("K_TR32R", "0")))    # f32r PE transposes

SCALE = float(D) ** -0.5
SA = 256.0             # attn scale (fp8 headroom)
SY = 16.0              # y_full scale in yf8
SW1X = 32.0            # W1 x-half scale (bf16)  == SW1Y*SY
SW1Y = 2.0             # W1 y-half scale (fp8)
SW2 = 32.0             # W2 scale (fp8)
LN256 = float(np.log(SA))

_prog_cache = {}


def _patch_act_tables(arch):
    """Make natural_log_exp_and_others the only provider of Exp/Ln so the
    compiler's table-load pass keeps one table across the attention loop.
    Mutates the functools-cached dict in place (names/ids unchanged)."""
    tabs = get_activation_tables(arch)
    keep = "natural_log_exp_and_others"
    if keep not in tabs:
        return
    for name, s in tabs.items():
        if name == keep:
            continue
        s.discard(AF.Exp)
        s.discard(AF.Ln)


def _build(g, ht, pt):
    omg = 1.0 - g
    f8 = dt.float8e4
    bf = dt.bfloat16
    f32 = dt.float32
    f32t = dt.float32r if TR32R else dt.float32

    nc = bacc.Bacc("TRN2", target_bir_lowering=False, debug=False, num_devices=8)
    _patch_act_tables(nc.m.arch)

    def mm_dr(out, l3, r3, start, stop):
        if not NODR:
            nc.tensor.matmul(out, l3, r3, start=start, stop=stop, perf_mode=DR)
        else:
            nc.tensor.matmul(out, l3[:, 0], r3[:, 0], start=start, stop=False)
            nc.tensor.matmul(out, l3[:, 1], r3[:, 1], start=False, stop=stop)

    # ---- DRAM params ----
    xtb_d = nc.declare_dram_parameter("xtb", [128, ND, N], bf, isOutput=False)
    yt_d = nc.declare_dram_parameter("yt", [128, ND, N], bf, isOutput=False)
    ybp_d = nc.declare_dram_parameter("ybp", [128, 4, 2, D], f8, isOutput=False)
    utb_d = nc.declare_dram_parameter("utb", [128, ND, ND, 128], bf, isOutput=False)
    posn_d = nc.declare_dram_parameter("posn", [128, NB, N], bf, isOutput=False)
    w1p_d = nc.declare_dram_parameter("w1p", [128, 3, NF, 2, 128], f8, isOutput=False)
    w1x_d = nc.declare_dram_parameter("w1x", [128, 3, NF, 2, 128], f8, isOutput=False)
    xf8_d = nc.declare_dram_parameter("xf8", [128, 3, 2, N], f8, isOutput=False)
    w2r_d = nc.declare_dram_parameter("w2r", [128, 12, 2, D], f8, isOutput=False)
    b1t_d = nc.declare_dram_parameter("b1t", [128, NF], f32, isOutput=False)
    b2r_d = nc.declare_dram_parameter("b2r", [1, D], bf, isOutput=False)
    s12_d = nc.declare_dram_parameter("s12", [128, 2, ND], f32, isOutput=False)
    xnat_d = nc.declare_dram_parameter("xnat", [N, D], f32, isOutput=False)
    out_d = nc.declare_dram_parameter("out", [N, D], f32, isOutput=True)
    hmbuf = nc.dram_tensor("hmbuf", [N, 1], f32)

    with tile.TileContext(nc) as tc:
        with tc.tile_pool(name="p0", bufs=1) as P0:
            # ---- persistent tiles ----
            w1p = P0.tile([128, 3, NF, 2, 128], f8, tag="w1p", name="w1p")
            w1x = P0.tile([128, 3, NF, 2, 128], f8, tag="w1x", name="w1x")
            xf8 = P0.tile([128, 3, 2, N], f8, tag="xf8", name="xf8")
            xtb = P0.tile([128, ND, N], bf, tag="xtb", name="xtb")
            yf8 = P0.tile([128, 3, 2, N], f8, tag="yf8", name="yf8")
            posn = P0.tile([128, NB, N], bf, tag="posn", name="posn")
            b2r = P0.tile([1, D], bf, tag="b2r", name="b2r")
            b1t = P0.tile([128, NF], f32, tag="b1t", name="b1t")
            s12 = P0.tile([128, 2, ND], f32, tag="s12", name="s12")
            identf = P0.tile([128, 128], f32, tag="identf", name="identf")
            identm = P0.tile([128, 128], bf, tag="identm", name="identm")
            onesr = P0.tile([1, 128], bf, tag="onesr", name="onesr")
            epsb = P0.tile([128, 1], f32, tag="epsb", name="epsb")
            zerop = P0.tile([128, 1], f32, tag="zerop", name="zerop")
            hbias = P0.tile([128, 1], f32, tag="hbias", name="hbias")

            # ---- gpsimd queue: s12 (phase-1 tail dep) + small inits ----
            nc.gpsimd.dma_start(s12[:], s12_d[:])
            make_identity(nc, identf[:])
            make_identity(nc, identm[:])
            nc.gpsimd.memset(epsb[:], SA * 1e-8)
            nc.gpsimd.memset(hbias[:], -ht * LN256)
            nc.gpsimd.memset(zerop[:], 0.0)
            nc.gpsimd.memset(onesr[:], 1.0)

            # ---- PE warmup: ramp the tensor engine to full clock while
            # the input DMAs stream (identm has no DMA dependency). ----
            with tc.tile_pool(name="pwu", bufs=1, space="PSUM") as PWU:
                wps = PWU.tile([128, 128], f32, tag="wps", name="wps")
                for _ in range(24):
                    nc.tensor.matmul(wps[:], identm[:], identm[:],
                                     start=True, stop=True)

            with tc.tile_pool(name="pa2", bufs=1) as PA2:
                acT = PA2.tile([128, 4, 2, N], f8, tag="acT", name="acT")
                ybp = PA2.tile([128, 4, 2, D], f8, tag="ybp", name="ybp")

                with tc.tile_pool(name="pa1", bufs=1) as PA1:
                    yUT = PA1.tile([128, ND, N], bf, tag="yUT", name="yUT")
                    ts0 = PA1.tile([128, ND, N], bf, tag="ts0", name="ts0")
                    ts1 = PA1.tile([128, ND, N], bf, tag="ts1", name="ts1")

                    # ---------- phase 1: tT, yUT ----------
                    with tc.tile_pool(name="p1", bufs=1) as P1, \
                         tc.tile_pool(name="ps1", bufs=4, space="PSUM") as PS1:
                        utb = P1.tile([128, ND, ND, 128], bf, tag="utb", name="utb")
                        yt = P1.tile([128, ND, N], bf, tag="yt", name="yt")
                        # phase-1-critical inputs first on their queues
                        for d in range(ND):
                            nc.scalar.dma_start(utb[:, d], utb_d[:, d])
                        for k in range(ND):
                            nc.sync.dma_start(xtb[:, k], xtb_d[:, k])
                        for k in range(ND):
                            nc.gpsimd.dma_start(yt[:, k], yt_d[:, k])
                        # first two posn blocks up front; rest stream
                        # just-in-time inside the phase-2 loop
                        for nb in range(2):
                            nc.sync.dma_start(posn[:, nb], posn_d[:, nb])

                        # k-outer over half the d's at a time: the first
                        # matmuls need only utb[d] + xtb[0], not all of xtb
                        for src, half in ((xtb, 0), (xtb, 1),
                                          (yt, 0), (yt, 1)):
                            ds = range(3 * half, 3 * half + 3)
                            pss = {}
                            for d in ds:
                                pss[d] = PS1.tile([128, N], f32, tag="psA",
                                                  name="psA")
                            for k in range(ND):
                                for d in ds:
                                    for h in range(2):
                                        nc.tensor.matmul(
                                            pss[d][:, 512 * h : 512 * h + 512],
                                            utb[:, d, k],
                                            src[:, k, 512 * h : 512 * h + 512],
                                            start=(k == 0), stop=(k == ND - 1),
                                        )
                            for d in ds:
                                if src is xtb:
                                    nc.vector.tensor_scalar_mul(
                                        ts0[:, d, :], pss[d][:],
                                        s12[:, 0, d : d + 1])
                                    nc.scalar.mul(ts1[:, d, :], pss[d][:],
                                                  s12[:, 1, d : d + 1])
                                else:
                                    nc.scalar.copy(yUT[:, d, :], pss[d][:])

                    # ---- phase 2: attention, entropy, routing ----
                    # Every engine queue is in-order, so the per-block chain
                    # exp -> combine -> ln -> mult -> reduce -> route -> dka
                    # -> transpose is software-pipelined three deep: loop
                    # step i emits S0(i), S1(i-1), S2(i-2). Each engine then
                    # always has ready work at its queue head.
                    with tc.tile_pool(name="pat", bufs=4) as PT, \
                         tc.tile_pool(name="plk", bufs=3) as LK, \
                         tc.tile_pool(name="pdk", bufs=4) as DK, \
                         tc.tile_pool(name="psm", bufs=8) as SM, \
                         tc.tile_pool(name="psl", bufs=3, space="PSUM") as PSL, \
                         tc.tile_pool(name="pstp", bufs=2, space="PSUM") as PST:

                        # big phase-3/4 weights stream on the sync queue
                        # during phase 2 (it only carries posn + heat stores)
                        w14_loads = (
                            [lambda i=i: nc.sync.dma_start(ybp[:, i], ybp_d[:, i])
                             for i in range(4)]
                            + [lambda: nc.sync.dma_start(b1t[:], b1t_d[:])]
                            + [lambda: nc.sync.dma_start(xf8[:], xf8_d[:])]
                            + [lambda i=i: nc.sync.dma_start(w1p[:, i], w1p_d[:, i])
                               for i in range(3)]
                            + [lambda i=i: nc.sync.dma_start(w1x[:, i], w1x_d[:, i])
                               for i in range(3)]
                        )

                        st_ = {}

                        def s0(nb):
                            # logits GEMMs, exp with fused row-sum, 1/esum
                            if 2 + nb < NB:
                                nc.sync.dma_start(posn[:, 2 + nb],
                                                  posn_d[:, 2 + nb])
                            for li in range(2 * nb, min(2 * nb + 2,
                                                        len(w14_loads))):
                                w14_loads[li]()
                            pk = PT.tile([128, 2, N], bf, tag="pk", name="pk")
                            rks = []
                            for k2 in range(2):
                                tsk = ts0 if k2 == 0 else ts1
                                psl = PSL.tile([128, N], f32, tag="psl",
                                               name="psl")
                                for e in range(ND):
                                    for h in range(2):
                                        nc.tensor.matmul(
                                            psl[:, 512 * h : 512 * h + 512],
                                            tsk[:, e, 128 * nb : 128 * nb + 128],
                                            yUT[:, e, 512 * h : 512 * h + 512],
                                            start=(e == 0), stop=(e == ND - 1),
                                        )
                                esum = SM.tile([128, 1], f32, tag="esum",
                                               name="esum")
                                nc.scalar.activation(pk[:, k2, :], psl[:],
                                                     AF.Exp, bias=zerop[:],
                                                     scale=SCALE,
                                                     accum_out=esum[:])
                                rkf = SM.tile([128, 1], f32, tag="rkf",
                                              name="rkf")
                                nc.vector.reciprocal(rkf[:], esum[:])
                                rk = SM.tile([128, 1], bf, tag="rk", name="rk")
                                nc.vector.tensor_scalar_mul(rk[:], rkf[:],
                                                            SA * omg)
                                rks.append(rk)
                            st_[("pk", nb)] = pk
                            st_[("rk", nb)] = rks

                        def s1(nb):
                            # pos combine, ln, entropy product
                            pk = st_[("pk", nb)]
                            rks = st_.pop(("rk", nb))
                            lnk = LK.tile([128, 2, N], bf, tag="lnk",
                                          name="lnk")
                            for k2 in range(2):
                                patch = pk[:, k2, :]
                                nc.vector.scalar_tensor_tensor(
                                    patch, patch, rks[k2][:], posn[:, nb, :],
                                    ALU.mult, ALU.add)
                            # single-pass ln and product over both k (fewer
                            # fixed overheads); product on DVE (all-bf16,
                            # avoids the shared DVE/Pool SBUF port)
                            nc.scalar.activation(lnk[:], pk[:], AF.Ln,
                                                 bias=epsb[:])
                            nc.vector.tensor_mul(lnk[:], lnk[:], pk[:])
                            d01 = DK.tile([128, N], bf, tag="d01", name="d01")
                            nc.gpsimd.tensor_sub(d01[:], pk[:, 0, :],
                                                 pk[:, 1, :])
                            st_[("lnk", nb)] = lnk
                            st_[("d01", nb)] = d01

                        def s2(nb):
                            # entropy reduce, routing, heat, dka, transposes
                            r0 = 128 * nb
                            pk = st_.pop(("pk", nb))
                            lnk = st_.pop(("lnk", nb))
                            d01 = st_.pop(("d01", nb))
                            accr = SM.tile([128, 2], f32, tag="accr",
                                           name="accr")
                            # entropy row-sums: k0 on DVE, k1 on ACT via
                            # Copy-with-accumulate (Copy is in every table)
                            nc.vector.tensor_reduce(
                                accr[:, 0:1], lnk[:, 0, :],
                                axis=mybir.AxisListType.X, op=ALU.add)
                            nc.scalar.activation(lnk[:, 1, :], lnk[:, 1, :],
                                                 AF.Copy, bias=0.0,
                                                 accum_out=accr[:, 1:2])
                            # route0 iff H0<=H1 iff accr0>=accr1
                            rsel = SM.tile([128, 1], bf, tag="rsel",
                                           name="rsel")
                            nc.vector.tensor_tensor(rsel[:], accr[:, 0:1],
                                                    accr[:, 1:2], ALU.is_ge)
                            amax = SM.tile([128, 1], f32, tag="amax",
                                           name="amax")
                            nc.vector.tensor_tensor(amax[:], accr[:, 0:1],
                                                    accr[:, 1:2], ALU.max)
                            # e = exp(-ht*H_sel) = exp(ht/256*amax - ht*ln256)
                            ee = SM.tile([128, 1], f32, tag="ee", name="ee")
                            nc.scalar.activation(ee[:], amax[:], AF.Exp,
                                                 scale=ht / SA, bias=hbias[:])
                            ep1 = SM.tile([128, 1], f32, tag="ep1", name="ep1")
                            nc.vector.tensor_scalar_add(ep1[:], ee[:], 1.0)
                            rcp = SM.tile([128, 1], f32, tag="rcp", name="rcp")
                            nc.vector.reciprocal(rcp[:], ep1[:])
                            # heat pre-scaled by 1/SW2 (W2 epilogue reads it
                            # as the only scale on the pso accumulator)
                            heat = SM.tile([128, 1], f32, tag="heat",
                                           name="heat")
                            nc.vector.scalar_tensor_tensor(
                                heat[:], ee[:], 2.0 / SW2, rcp[:],
                                ALU.mult, ALU.mult)
                            nc.sync.dma_start(hmbuf[r0 : r0 + 128, 0:1],
                                              heat[:])
                            dka = DK.tile([128, N], f32t, tag="dka",
                                          name="dka")
                            nc.vector.scalar_tensor_tensor(
                                dka[:], d01[:], rsel[:], pk[:, 1, :],
                                ALU.mult, ALU.add)
                            # 8 transposes into two 512-wide PSUM tiles, one
                            # batched fp8 copy per group of 4
                            for grp in range(2):
                                pst = PST.tile([128, 512], f32t, tag="pst",
                                               name="pst")
                                for q in range(4):
                                    mb = 4 * grp + q
                                    nc.tensor.transpose(
                                        pst[:, 128 * q : 128 * q + 128],
                                        dka[:, 128 * mb : 128 * mb + 128],
                                        identf[:])
                                dst = acT[:, 2 * grp : 2 * grp + 2, :,
                                          r0 : r0 + 128]
                                nc.scalar.copy(dst, pst[:])

                        for i in range(NB + 2):
                            if i < NB:
                                s0(i)
                            if 1 <= i <= NB:
                                s1(i - 1)
                            if i >= 2:
                                s2(i - 2)

                # ---------- phase 3: y_outT (fp8 DoubleRow) -> yf8 ----------
                with tc.tile_pool(name="psy", bufs=2, space="PSUM") as PSY:
                    for d in range(ND):
                        psy = PSY.tile([128, N], f32, tag="psy", name="psy")
                        for mbp in range(4):
                            for h in range(2):
                                mm_dr(
                                    psy[:, 512 * h : 512 * h + 512],
                                    ybp[:, mbp, :, 128 * d : 128 * d + 128],
                                    acT[:, mbp, :, 512 * h : 512 * h + 512],
                                    (mbp == 0), (mbp == 3),
                                )
                        if d % 2 == 0:
                            nc.scalar.mul(yf8[:, d // 2, d % 2, :],
                                          psy[:], SY / SA)
                        else:
                            nc.vector.tensor_scalar_mul(
                                yf8[:, d // 2, d % 2, :], psy[:], SY / SA)

            # ---------- phase 4: MLP ----------
            with tc.tile_pool(name="pg", bufs=1) as PG:
                w2r = PG.tile([128, 12, 2, D], f8, tag="w2r", name="w2r")
                gel = PG.tile([128, 12, 2, N], f8, tag="gel", name="gel")
                nc.sync.dma_start(b2r[:], b2r_d[:])
                for q in range(4):
                    nc.sync.dma_start(w2r[:, 3 * q : 3 * q + 3],
                                      w2r_d[:, 3 * q : 3 * q + 3])

                chunksA = [(0, 512), (512, 512)]
                with tc.tile_pool(name="psh", bufs=2, space="PSUM") as PSH:
                    for f in range(NF):
                        psh = PSH.tile([128, N], f32, tag="psh", name="psh")
                        for xp in range(3):
                            for (s0, wd) in chunksA:
                                mm_dr(
                                    psh[:, s0 : s0 + wd],
                                    w1x[:, xp, f],
                                    xf8[:, xp, :, s0 : s0 + wd],
                                    (xp == 0), False,
                                )
                        for yp in range(3):
                            for (s0, wd) in chunksA:
                                mm_dr(
                                    psh[:, s0 : s0 + wd],
                                    w1p[:, yp, f],
                                    yf8[:, yp, :, s0 : s0 + wd],
                                    False, (yp == 2),
                                )
                        nc.scalar.activation(gel[:, f // 2, f % 2, :], psh[:],
                                             AF.Gelu, bias=b1t[:, f : f + 1],
                                             scale=1.0 / SW1X)

                with tc.tile_pool(name="p5", bufs=3) as P5, \
                     tc.tile_pool(name="pso", bufs=2, space="PSUM") as PSO:
                    chunksB = [(512, D - 512), (0, 512)]
                    for tb in range(NB):
                        r0 = 128 * tb
                        xn = P5.tile([128, D], f32, tag="xn", name="xn")
                        nc.sync.dma_start(xn[:], xnat_d[r0 : r0 + 128, :])
                        hmc = P5.tile([128, 1], f32, tag="hmc", name="hmc")
                        nc.sync.dma_start(hmc[:], hmbuf[r0 : r0 + 128, 0:1])
                        pso = PSO.tile([128, D], f32, tag="pso", name="pso")
                        for fp in range(12):
                            for (s0, wd) in chunksB:
                                mm_dr(
                                    pso[:, s0 : s0 + wd],
                                    gel[:, fp, :, r0 : r0 + 128],
                                    w2r[:, fp, :, s0 : s0 + wd],
                                    (fp == 0), False,
                                )
                        # rank-1 b2 bias fold: pso += ones.T @ (SW2*b2)
                        for (s0, wd) in chunksB:
                            nc.tensor.matmul(
                                pso[:, s0 : s0 + wd],
                                onesr[0:1, 0:128],
                                b2r[0:1, s0 : s0 + wd],
                                start=False, stop=True,
                            )
                        # single drain op: out = pso*(heat/SW2) + x
                        ot = P5.tile([128, D], f32, tag="ot", name="ot")
                        nc.vector.scalar_tensor_tensor(
                            ot[:], pso[:], hmc[:], xn[:],
                            ALU.mult, ALU.add)
                        nc.sync.dma_start(out_d[r0 : r0 + 128, :], ot[:])

    nc.compile()
    return nc


def _get_prog(g, ht, pt):
    key = (round(float(g), 9), round(float(ht), 9), round(float(pt), 9))
    if key not in _prog_cache:
        _prog_cache[key] = _build(*key)
    return _prog_cache[key]


def kernel(x, y, coords, U, S1, S2, gating, h_temp, p_temp, pos_emb, W1, b1, W2, b2):
    x = np.asarray(x, dtype=np.float32)
    y = np.asarray(y, dtype=np.float32)
    coords = np.asarray(coords, dtype=np.float32)
    U = np.asarray(U, dtype=np.float32)
    bf16 = ml_dtypes.bfloat16
    f8 = ml_dtypes.float8_e4m3

    g = float(1.0 / (1.0 + np.exp(-float(np.asarray(gating)))))
    ht = float(np.asarray(h_temp))
    pt = abs(float(np.asarray(p_temp)))
    nc = _get_prog(g, ht, pt)

    def q8(a):
        return np.clip(a, -240.0, 240.0).astype(f8)

    # ---- shared (replicated) host prep ----
    UT = np.ascontiguousarray(U.T)
    utb = np.ascontiguousarray(
        UT.reshape(ND, 128, ND, 128).transpose(1, 2, 0, 3)).astype(bf16)
    s12 = np.ascontiguousarray(np.stack(
        [np.abs(np.asarray(S1, np.float32)).reshape(ND, 128).T,
         np.abs(np.asarray(S2, np.float32)).reshape(ND, 128).T], axis=1))
    # pos softmax on host (exact):
    #   posn[i, j] = SA*g*softmax_j(-pt * sum_c coords[i,j,c]*pe[i,c])
    pe_f = np.asarray(pos_emb, np.float32)[:, :, 0]            # [N, 6]
    pl = -pt * np.einsum("ijc,ic->ij", coords, pe_f)           # [N, N]
    pl -= pl.max(axis=1, keepdims=True)
    pexp = np.exp(pl)
    psm = pexp / pexp.sum(axis=1, keepdims=True)
    posn = np.ascontiguousarray(
        (SA * g * psm).reshape(NB, 128, N).transpose(1, 0, 2)).astype(bf16)
    W1 = np.asarray(W1, np.float32)
    W1a, W1b = W1[:D], W1[D:]
    w1x = q8((SW1Y * W1a).reshape(3, 2, 128, NF, 128).transpose(2, 0, 3, 1, 4))
    w1p = q8((SW1Y * W1b).reshape(3, 2, 128, NF, 128).transpose(2, 0, 3, 1, 4))
    W2 = np.asarray(W2, np.float32)
    w2r = q8((SW2 * W2).reshape(12, 2, 128, D).transpose(2, 0, 1, 3))
    b1 = np.asarray(b1, np.float32)
    b2 = np.asarray(b2, np.float32)
    b1t = np.ascontiguousarray(b1.reshape(NF, 128).T)
    b2r = np.ascontiguousarray((SW2 * b2).reshape(1, D)).astype(bf16)

    shared = {"utb": utb, "s12": s12, "posn": posn,
              "w1x": w1x, "w1p": w1p, "w2r": w2r, "b1t": b1t, "b2r": b2r}

    in_maps = []
    cls_rows = []
    for b in range(B):
        xb = x[b, 1:]
        yb = y[b, 1:]
        xtb = np.ascontiguousarray(
            xb.T.reshape(ND, 128, N).transpose(1, 0, 2)).astype(bf16)
        yt = np.ascontiguousarray(
            yb.T.reshape(ND, 128, N).transpose(1, 0, 2)).astype(bf16)
        ybp = q8(yb.reshape(4, 2, 128, D).transpose(2, 0, 1, 3))
        xf8 = q8(SY * xb.T.reshape(3, 2, 128, N).transpose(2, 0, 1, 3))
        # CLS output row on host (exact f32 1x1536 MLP; no heat scaling)
        h0 = np.concatenate([x[b, 0], y[b, 0]]) @ W1 + b1
        xp0 = (h0 * 0.5 * (1.0 + erf(h0 / np.sqrt(2.0)))) @ W2 + b2
        cls_rows.append(x[b, 0] + xp0)
        m = dict(shared)
        m["xtb"] = xtb
        m["yt"] = yt
        m["ybp"] = ybp
        m["xf8"] = xf8
        m["xnat"] = np.ascontiguousarray(xb)
        in_maps.append(m)

    res = run_bass_kernel_spmd(nc, in_maps, list(range(B)))
    out = np.empty((B, N + 1, D), np.float32)
    for b in range(B):
        out[b, 0] = cls_rows[b]
        out[b, 1:] = res.results[b]["out"]
    return out


if __name__ == "__main__":
    import time
    sys.path.insert(0, "/root/problem")
    from reference import setup_inputs, reference

    inp = {k: np.asarray(v) for k, v in setup_inputs().items()}
    t0 = time.time()
    got = kernel(**inp)
    print("kernel wall:", time.time() - t0)
    exp = np.asarray(reference(**inp))
    d = np.abs(got - exp)
    print("absmax_rel:", d.max() / np.abs(exp).max())
    print("rms_rel:", np.sqrt((d ** 2).mean()) / np.sqrt((exp ** 2).mean()))


# revision 66
# speedup vs baseline: 1.3498x; 1.0942x over previous
"""Trainium2 Bass kernel for nn_DepthCueRectification_Sp.

Data-parallel over batch: 8 batch elements -> 8 NeuronCores (SPMD).

Host precomputes (exact, f32): pos softmax (posn = SA*g*softmax(pos_logits)),
the CLS output row (tiny 1x1536 MLP), and all weight/layout prep. The device
handles only the N=1024 patch tokens - no padding, no CLS special cases.

Per-core pipeline (D=768, N=1024):
  tT    = U @ xb.T                  (bf16)
  yUT   = U @ yb.T                  (bf16)   [algebra: logits_k =
                                     (|S_k|*t) @ (y@U.T).T]
  tsT_k = |S_k|-scaled copies of tT (ACT per-partition scale)
  logits_k -> exp (no max-sub, fused row-sum) -> attn_k = 256*attn (bf16)
  patch_k += posn (host-exact pos term, bf16)
  entropy: Ln on ACT; fused mult+row-sum via DVE scalar_tensor_tensor
  accum_out; routing compares raw accums; heat = 2e/(1+e) via one exp
  dka (selected attn, f32r) -> PE transpose -> acT fp8 pairs
  y_outT = fp8 DoubleRow (ybp pairs @ acT pairs) -> yf8 = 16*y_out.T fp8
  MLP W1: x-half bf16 (xtb @ 32*W1a) + y-half fp8 DR (yf8 @ 2*W1b) = 32*h
  gel   = gelu(psh/32 + b1) -> fp8 pairs
  MLP W2: fp8 DoubleRow (gel pairs @ 32*W2 pairs) = 32*xp'
  out   = x + heat*(xp' + b2)

The act-table dict is patched so Exp and Ln resolve to the combined
natural_log_exp_and_others set (avoids per-iteration table reloads).
tensor_tensor_reduce and 16-bit PE transposes hard-crash the exec unit
on this toolchain and are not used.
"""

import os
import sys

if "/opt/trn_rl_repo" not in sys.path:
    sys.path.insert(0, "/opt/trn_rl_repo")

import numpy as np
import ml_dtypes
from scipy.special import erf

import concourse.bass as bass
import concourse.bass_utils as _bu
import concourse.mybir as mybir
import concourse.tile as tile
from concourse import bacc
from concourse.bass_utils import run_bass_kernel_spmd
from concourse.hw_specs import get_activation_tables
from concourse.masks import make_identity

B, N, D, DFF = 8, 1024, 768, 3072
ND = D // 128          # 6
NB = N // 128          # 8
NF = DFF // 128        # 24
AF = mybir.ActivationFunctionType
ALU = mybir.AluOpType
dt = mybir.dt
DR = mybir.MatmulPerfMode.DoubleRow

NODR = bool(int(os.environ.get("K_NODR", "0")))      # disable DoubleRow
TR32R = bool(int(os.environ.get("K_TR32R", "0")))    # f32r PE transposes

SCALE = float(D) ** -0.5
SA = 256.0             # attn scale (fp8 headroom)
SY = 16.0              # y_full scale in yf8
SW1X = 32.0            # W1 x-half scale (bf16)  == SW1Y*SY
SW1Y = 2.0             # W1 y-half scale (fp8)
SW2 = 32.0             # W2 scale (fp8)
SU8 = 32.0             # U scale (fp8, phase 1)
LN256 = float(np.log(SA))

_prog_cache = {}


def _patch_act_tables(arch):
    """Make natural_log_exp_and_others the only provider of Exp/Ln so the
    compiler's table-load pass keeps one table across the attention loop.
    Mutates the functools-cached dict in place (names/ids unchanged)."""
    tabs = get_activation_tables(arch)
    keep = "natural_log_exp_and_others"
    if keep not in tabs:
        return
    for name, s in tabs.items():
        if name == keep:
            continue
        s.discard(AF.Exp)
        s.discard(AF.Ln)


def _build(g, ht, pt):
    omg = 1.0 - g
    f8 = dt.float8e4
    bf = dt.bfloat16
    f32 = dt.float32
    f32t = dt.float32r if TR32R else dt.float32

    nc = bacc.Bacc("TRN2", target_bir_lowering=False, debug=False, num_devices=8)
    _patch_act_tables(nc.m.arch)

    def mm_dr(out, l3, r3, start, stop):
        if not NODR:
            nc.tensor.matmul(out, l3, r3, start=start, stop=stop, perf_mode=DR)
        else:
            nc.tensor.matmul(out, l3[:, 0], r3[:, 0], start=start, stop=False)
            nc.tensor.matmul(out, l3[:, 1], r3[:, 1], start=False, stop=stop)

    # ---- DRAM params ----
    ybt8_d = nc.declare_dram_parameter("ybt8", [128, 3, 2, N], f8, isOutput=False)
    ybp_d = nc.declare_dram_parameter("ybp", [128, 4, 2, D], f8, isOutput=False)
    utb_d = nc.declare_dram_parameter("utb", [128, ND, 3, 2, 128], f8, isOutput=False)
    posn_d = nc.declare_dram_parameter("posn", [128, NB, N], bf, isOutput=False)
    w1p_d = nc.declare_dram_parameter("w1p", [128, 3, NF, 2, 128], f8, isOutput=False)
    w1x_d = nc.declare_dram_parameter("w1x", [128, 3, NF, 2, 128], f8, isOutput=False)
    xf8_d = nc.declare_dram_parameter("xf8", [128, 3, 2, N], f8, isOutput=False)
    w2r_d = nc.declare_dram_parameter("w2r", [128, 12, 2, D], f8, isOutput=False)
    b1t_d = nc.declare_dram_parameter("b1t", [128, NF], f32, isOutput=False)
    b2r_d = nc.declare_dram_parameter("b2r", [1, D], bf, isOutput=False)
    s12_d = nc.declare_dram_parameter("s12", [128, 2, ND], f32, isOutput=False)
    xnat_d = nc.declare_dram_parameter("xnat", [N, D], f32, isOutput=False)
    out_d = nc.declare_dram_parameter("out", [N, D], f32, isOutput=True)
    hmbuf = nc.dram_tensor("hmbuf", [N, 1], f32)

    with tile.TileContext(nc) as tc:
        with tc.tile_pool(name="p0", bufs=1) as P0:
            # ---- persistent tiles ----
            w1p = P0.tile([128, 3, NF, 2, 128], f8, tag="w1p", name="w1p")
            w1x = P0.tile([128, 3, NF, 2, 128], f8, tag="w1x", name="w1x")
            xf8 = P0.tile([128, 3, 2, N], f8, tag="xf8", name="xf8")
            yf8 = P0.tile([128, 3, 2, N], f8, tag="yf8", name="yf8")
            posn = P0.tile([128, NB, N], bf, tag="posn", name="posn")
            b2r = P0.tile([1, D], bf, tag="b2r", name="b2r")
            b1t = P0.tile([128, NF], f32, tag="b1t", name="b1t")
            s12 = P0.tile([128, 2, ND], f32, tag="s12", name="s12")
            identf = P0.tile([128, 128], f32, tag="identf", name="identf")
            identm = P0.tile([128, 128], bf, tag="identm", name="identm")
            onesr = P0.tile([1, 128], bf, tag="onesr", name="onesr")
            epsb = P0.tile([128, 1], f32, tag="epsb", name="epsb")
            zerop = P0.tile([128, 1], f32, tag="zerop", name="zerop")
            hbias = P0.tile([128, 1], f32, tag="hbias", name="hbias")

            # ---- gpsimd queue: s12 (phase-1 tail dep) + small inits ----
            nc.gpsimd.dma_start(s12[:], s12_d[:])
            make_identity(nc, identf[:])
            make_identity(nc, identm[:])
            nc.gpsimd.memset(epsb[:], SA * 1e-8)
            nc.gpsimd.memset(hbias[:], -ht * LN256)
            nc.gpsimd.memset(zerop[:], 0.0)
            nc.gpsimd.memset(onesr[:], 1.0)

            # ---- PE warmup: ramp the tensor engine to full clock while
            # the input DMAs stream (identm has no DMA dependency). ----
            with tc.tile_pool(name="pwu", bufs=1, space="PSUM") as PWU:
                wps = PWU.tile([128, 128], f32, tag="wps", name="wps")
                for _ in range(24):
                    nc.tensor.matmul(wps[:], identm[:], identm[:],
                                     start=True, stop=True)

            with tc.tile_pool(name="pa2", bufs=1) as PA2:
                acT = PA2.tile([128, 4, 2, N], f8, tag="acT", name="acT")
                ybp = PA2.tile([128, 4, 2, D], f8, tag="ybp", name="ybp")

                with tc.tile_pool(name="pa1", bufs=1) as PA1:
                    yUT = PA1.tile([128, ND, N], bf, tag="yUT", name="yUT")
                    ts0 = PA1.tile([128, ND, N], bf, tag="ts0", name="ts0")
                    ts1 = PA1.tile([128, ND, N], bf, tag="ts1", name="ts1")

                    # ---------- phase 1: tT, yUT (fp8 DoubleRow) ----------
                    with tc.tile_pool(name="p1", bufs=1) as P1, \
                         tc.tile_pool(name="ps1", bufs=4, space="PSUM") as PS1:
                        utb = P1.tile([128, ND, 3, 2, 128], f8, tag="utb",
                                      name="utb")
                        ybt8 = P1.tile([128, 3, 2, N], f8, tag="ybt8",
                                       name="ybt8")
                        # phase-1-critical inputs first on their queues
                        for d in range(ND):
                            nc.scalar.dma_start(utb[:, d], utb_d[:, d])
                        nc.sync.dma_start(xf8[:], xf8_d[:])
                        nc.gpsimd.dma_start(ybt8[:], ybt8_d[:])
                        # first two posn blocks up front; rest stream
                        # just-in-time inside the phase-2 loop
                        for nb in range(2):
                            nc.sync.dma_start(posn[:, nb], posn_d[:, nb])

                        # k-outer over half the d's at a time: the first
                        # matmuls need only utb[d] + the first src chunk
                        for src, half in ((xf8, 0), (xf8, 1),
                                          (ybt8, 0), (ybt8, 1)):
                            ds = range(3 * half, 3 * half + 3)
                            pss = {}
                            for d in ds:
                                pss[d] = PS1.tile([128, N], f32, tag="psA",
                                                  name="psA")
                            for k in range(3):
                                for d in ds:
                                    for h in range(2):
                                        mm_dr(
                                            pss[d][:, 512 * h : 512 * h + 512],
                                            utb[:, d, k],
                                            src[:, k, :,
                                                512 * h : 512 * h + 512],
                                            (k == 0), (k == 2),
                                        )
                            for d in ds:
                                if src is xf8:
                                    nc.vector.tensor_scalar_mul(
                                        ts0[:, d, :], pss[d][:],
                                        s12[:, 0, d : d + 1])
                                    nc.scalar.mul(ts1[:, d, :], pss[d][:],
                                                  s12[:, 1, d : d + 1])
                                else:
                                    nc.scalar.mul(yUT[:, d, :], pss[d][:],
                                                  1.0 / (SU8 * SY))

                    # ---- phase 2: attention, entropy, routing ----
                    # Every engine queue is in-order, so the per-block chain
                    # exp -> combine -> ln -> mult -> reduce -> route -> dka
                    # -> transpose is software-pipelined three deep: loop
                    # step i emits S0(i), S1(i-1), S2(i-2). Each engine then
                    # always has ready work at its queue head.
                    with tc.tile_pool(name="pat", bufs=4) as PT, \
                         tc.tile_pool(name="plk", bufs=3) as LK, \
                         tc.tile_pool(name="pdk", bufs=4) as DK, \
                         tc.tile_pool(name="psm", bufs=8) as SM, \
                         tc.tile_pool(name="psl", bufs=3, space="PSUM") as PSL, \
                         tc.tile_pool(name="pstp", bufs=2, space="PSUM") as PST:

                        # big phase-3/4 weights stream on the sync queue
                        # during phase 2 (it only carries posn + heat stores)
                        w14_loads = (
                            [lambda i=i: nc.sync.dma_start(ybp[:, i], ybp_d[:, i])
                             for i in range(4)]
                            + [lambda: nc.sync.dma_start(b1t[:], b1t_d[:])]
                            + [lambda i=i: nc.sync.dma_start(w1p[:, i], w1p_d[:, i])
                               for i in range(3)]
                            + [lambda i=i: nc.sync.dma_start(w1x[:, i], w1x_d[:, i])
                               for i in range(3)]
                        )

                        st_ = {}

                        def s0(nb):
                            # logits GEMMs, exp with fused row-sum, 1/esum
                            if 2 + nb < NB:
                                nc.sync.dma_start(posn[:, 2 + nb],
                                                  posn_d[:, 2 + nb])
                            for li in range(2 * nb, min(2 * nb + 2,
                                                        len(w14_loads))):
                                w14_loads[li]()
                            pk = PT.tile([128, 2, N], bf, tag="pk", name="pk")
                            rks = []
                            for k2 in range(2):
                                tsk = ts0 if k2 == 0 else ts1
                                psl = PSL.tile([128, N], f32, tag="psl",
                                               name="psl")
                                for e in range(ND):
                                    for h in range(2):
                                        nc.tensor.matmul(
                                            psl[:, 512 * h : 512 * h + 512],
                                            tsk[:, e, 128 * nb : 128 * nb + 128],
                                            yUT[:, e, 512 * h : 512 * h + 512],
                                            start=(e == 0), stop=(e == ND - 1),
                                        )
                                esum = SM.tile([128, 1], f32, tag="esum",
                                               name="esum")
                                nc.scalar.activation(pk[:, k2, :], psl[:],
                                                     AF.Exp, bias=zerop[:],
                                                     scale=SCALE,
                                                     accum_out=esum[:])
                                rkf = SM.tile([128, 1], f32, tag="rkf",
                                              name="rkf")
                                nc.vector.reciprocal(rkf[:], esum[:])
                                rk = SM.tile([128, 1], bf, tag="rk", name="rk")
                                nc.vector.tensor_scalar_mul(rk[:], rkf[:],
                                                            SA * omg)
                                rks.append(rk)
                            st_[("pk", nb)] = pk
                            st_[("rk", nb)] = rks

                        def s1(nb):
                            # pos combine, ln, entropy product
                            pk = st_[("pk", nb)]
                            rks = st_.pop(("rk", nb))
                            lnk = LK.tile([128, 2, N], bf, tag="lnk",
                                          name="lnk")
                            for k2 in range(2):
                                patch = pk[:, k2, :]
                                nc.vector.scalar_tensor_tensor(
                                    patch, patch, rks[k2][:], posn[:, nb, :],
                                    ALU.mult, ALU.add)
                            # single-pass ln and product over both k (fewer
                            # fixed overheads); product on DVE (all-bf16,
                            # avoids the shared DVE/Pool SBUF port)
                            nc.scalar.activation(lnk[:], pk[:], AF.Ln,
                                                 bias=epsb[:])
                            nc.vector.tensor_mul(lnk[:], lnk[:], pk[:])
                            d01 = DK.tile([128, N], bf, tag="d01", name="d01")
                            nc.gpsimd.tensor_sub(d01[:], pk[:, 0, :],
                                                 pk[:, 1, :])
                            st_[("lnk", nb)] = lnk
                            st_[("d01", nb)] = d01

                        def s2(nb):
                            # entropy reduce, routing, heat, dka, transposes
                            r0 = 128 * nb
                            pk = st_.pop(("pk", nb))
                            lnk = st_.pop(("lnk", nb))
                            d01 = st_.pop(("d01", nb))
                            accr = SM.tile([128, 2], f32, tag="accr",
                                           name="accr")
                            # entropy row-sums: k0 on DVE, k1 on ACT via
                            # Copy-with-accumulate (Copy is in every table)
                            nc.vector.tensor_reduce(
                                accr[:, 0:1], lnk[:, 0, :],
                                axis=mybir.AxisListType.X, op=ALU.add)
                            nc.scalar.activation(lnk[:, 1, :], lnk[:, 1, :],
                                                 AF.Copy, bias=0.0,
                                                 accum_out=accr[:, 1:2])
                            # route0 iff H0<=H1 iff accr0>=accr1
                            rsel = SM.tile([128, 1], bf, tag="rsel",
                                           name="rsel")
                            nc.vector.tensor_tensor(rsel[:], accr[:, 0:1],
                                                    accr[:, 1:2], ALU.is_ge)
                            amax = SM.tile([128, 1], f32, tag="amax",
                                           name="amax")
                            nc.vector.tensor_tensor(amax[:], accr[:, 0:1],
                                                    accr[:, 1:2], ALU.max)
                            # e = exp(-ht*H_sel) = exp(ht/256*amax - ht*ln256)
                            ee = SM.tile([128, 1], f32, tag="ee", name="ee")
                            nc.scalar.activation(ee[:], amax[:], AF.Exp,
                                                 scale=ht / SA, bias=hbias[:])
                            ep1 = SM.tile([128, 1], f32, tag="ep1", name="ep1")
                            nc.vector.tensor_scalar_add(ep1[:], ee[:], 1.0)
                            rcp = SM.tile([128, 1], f32, tag="rcp", name="rcp")
                            nc.vector.reciprocal(rcp[:], ep1[:])
                            # heat pre-scaled by 1/SW2 (W2 epilogue reads it
                            # as the only scale on the pso accumulator)
                            heat = SM.tile([128, 1], f32, tag="heat",
                                           name="heat")
                            nc.vector.scalar_tensor_tensor(
                                heat[:], ee[:], 2.0 / SW2, rcp[:],
                                ALU.mult, ALU.mult)
                            nc.sync.dma_start(hmbuf[r0 : r0 + 128, 0:1],
                                              heat[:])
                            dka = DK.tile([128, N], f32t, tag="dka",
                                          name="dka")
                            nc.vector.scalar_tensor_tensor(
                                dka[:], d01[:], rsel[:], pk[:, 1, :],
                                ALU.mult, ALU.add)
                            # 8 transposes into two 512-wide PSUM tiles, one
                            # batched fp8 copy per group of 4
                            for grp in range(2):
                                pst = PST.tile([128, 512], f32t, tag="pst",
                                               name="pst")
                                for q in range(4):
                                    mb = 4 * grp + q
                                    nc.tensor.transpose(
                                        pst[:, 128 * q : 128 * q + 128],
                                        dka[:, 128 * mb : 128 * mb + 128],
                                        identf[:])
                                dst = acT[:, 2 * grp : 2 * grp + 2, :,
                                          r0 : r0 + 128]
                                nc.scalar.copy(dst, pst[:])

                        for i in range(NB + 2):
                            if i < NB:
                                s0(i)
                            if 1 <= i <= NB:
                                s1(i - 1)
                            if i >= 2:
                                s2(i - 2)

                # ---------- phase 3: y_outT (fp8 DoubleRow) -> yf8 ----------
                with tc.tile_pool(name="psy", bufs=2, space="PSUM") as PSY:
                    for d in range(ND):
                        psy = PSY.tile([128, N], f32, tag="psy", name="psy")
                        for mbp in range(4):
                            for h in range(2):
                                mm_dr(
                                    psy[:, 512 * h : 512 * h + 512],
                                    ybp[:, mbp, :, 128 * d : 128 * d + 128],
                                    acT[:, mbp, :, 512 * h : 512 * h + 512],
                                    (mbp == 0), (mbp == 3),
                                )
                        if d % 2 == 0:
                            nc.scalar.mul(yf8[:, d // 2, d % 2, :],
                                          psy[:], SY / SA)
                        else:
                            nc.vector.tensor_scalar_mul(
                                yf8[:, d // 2, d % 2, :], psy[:], SY / SA)

            # ---------- phase 4: MLP ----------
            with tc.tile_pool(name="pg", bufs=1) as PG:
                w2r = PG.tile([128, 12, 2, D], f8, tag="w2r", name="w2r")
                gel = PG.tile([128, 12, 2, N], f8, tag="gel", name="gel")
                nc.sync.dma_start(b2r[:], b2r_d[:])
                for q in range(4):
                    nc.sync.dma_start(w2r[:, 3 * q : 3 * q + 3],
                                      w2r_d[:, 3 * q : 3 * q + 3])

                chunksA = [(0, 512), (512, 512)]
                with tc.tile_pool(name="psh", bufs=2, space="PSUM") as PSH:
                    for f in range(NF):
                        psh = PSH.tile([128, N], f32, tag="psh", name="psh")
                        for xp in range(3):
                            for (s0, wd) in chunksA:
                                mm_dr(
                                    psh[:, s0 : s0 + wd],
                                    w1x[:, xp, f],
                                    xf8[:, xp, :, s0 : s0 + wd],
                                    (xp == 0), False,
                                )
                        for yp in range(3):
                            for (s0, wd) in chunksA:
                                mm_dr(
                                    psh[:, s0 : s0 + wd],
                                    w1p[:, yp, f],
                                    yf8[:, yp, :, s0 : s0 + wd],
                                    False, (yp == 2),
                                )
                        nc.scalar.activation(gel[:, f // 2, f % 2, :], psh[:],
                                             AF.Gelu, bias=b1t[:, f : f + 1],
                                             scale=1.0 / SW1X)

                with tc.tile_pool(name="p5", bufs=3) as P5, \
                     tc.tile_pool(name="pso", bufs=2, space="PSUM") as PSO:
                    chunksB = [(512, D - 512), (0, 512)]
                    for tb in range(NB):
                        r0 = 128 * tb
                        xn = P5.tile([128, D], f32, tag="xn", name="xn")
                        nc.sync.dma_start(xn[:], xnat_d[r0 : r0 + 128, :])
                        hmc = P5.tile([128, 1], f32, tag="hmc", name="hmc")
                        nc.sync.dma_start(hmc[:], hmbuf[r0 : r0 + 128, 0:1])
                        pso = PSO.tile([128, D], f32, tag="pso", name="pso")
                        for fp in range(12):
                            for (s0, wd) in chunksB:
                                mm_dr(
                                    pso[:, s0 : s0 + wd],
                                    gel[:, fp, :, r0 : r0 + 128],
                                    w2r[:, fp, :, s0 : s0 + wd],
                                    (fp == 0), False,
                                )
                        # rank-1 b2 bias fold: pso += ones.T @ (SW2*b2)
                        for (s0, wd) in chunksB:
                            nc.tensor.matmul(
                                pso[:, s0 : s0 + wd],
                                onesr[0:1, 0:128],
                                b2r[0:1, s0 : s0 + wd],
                                start=False, stop=True,
                            )
                        # single drain op: out = pso*(heat/SW2) + x
                        ot = P5.tile([128, D], f32, tag="ot", name="ot")
                        nc.vector.scalar_tensor_tensor(
                            ot[:], pso[:], hmc[:], xn[:],
                            ALU.mult, ALU.add)
                        nc.sync.dma_start(out_d[r0 : r0 + 128, :], ot[:])

    nc.compile()
    return nc


def _get_prog(g, ht, pt):
    key = (round(float(g), 9), round(float(ht), 9), round(float(pt), 9))
    if key not in _prog_cache:
        _prog_cache[key] = _build(*key)
    return _prog_cache[key]


def kernel(x, y, coords, U, S1, S2, gating, h_temp, p_temp, pos_emb, W1, b1, W2, b2):
    x = np.asarray(x, dtype=np.float32)
    y = np.asarray(y, dtype=np.float32)
    coords = np.asarray(coords, dtype=np.float32)
    U = np.asarray(U, dtype=np.float32)
    bf16 = ml_dtypes.bfloat16
    f8 = ml_dtypes.float8_e4m3

    g = float(1.0 / (1.0 + np.exp(-float(np.asarray(gating)))))
    ht = float(np.asarray(h_temp))
    pt = abs(float(np.asarray(p_temp)))
    nc = _get_prog(g, ht, pt)

    def q8(a):
        return np.clip(a, -240.0, 240.0).astype(f8)

    # ---- shared (replicated) host prep ----
    UT = np.ascontiguousarray(U.T)
    utb = q8((SU8 * UT).reshape(3, 2, 128, ND, 128).transpose(2, 3, 0, 1, 4))
    # phase-1 psum carries SU8*SY; fold the un-scale into s12
    s12 = np.ascontiguousarray(np.stack(
        [np.abs(np.asarray(S1, np.float32)).reshape(ND, 128).T,
         np.abs(np.asarray(S2, np.float32)).reshape(ND, 128).T],
        axis=1)) / (SU8 * SY)
    # pos softmax on host (exact):
    #   posn[i, j] = SA*g*softmax_j(-pt * sum_c coords[i,j,c]*pe[i,c])
    pe_f = np.asarray(pos_emb, np.float32)[:, :, 0]            # [N, 6]
    pl = -pt * np.einsum("ijc,ic->ij", coords, pe_f)           # [N, N]
    pl -= pl.max(axis=1, keepdims=True)
    pexp = np.exp(pl)
    psm = pexp / pexp.sum(axis=1, keepdims=True)
    posn = np.ascontiguousarray(
        (SA * g * psm).reshape(NB, 128, N).transpose(1, 0, 2)).astype(bf16)
    W1 = np.asarray(W1, np.float32)
    W1a, W1b = W1[:D], W1[D:]
    w1x = q8((SW1Y * W1a).reshape(3, 2, 128, NF, 128).transpose(2, 0, 3, 1, 4))
    w1p = q8((SW1Y * W1b).reshape(3, 2, 128, NF, 128).transpose(2, 0, 3, 1, 4))
    W2 = np.asarray(W2, np.float32)
    w2r = q8((SW2 * W2).reshape(12, 2, 128, D).transpose(2, 0, 1, 3))
    b1 = np.asarray(b1, np.float32)
    b2 = np.asarray(b2, np.float32)
    b1t = np.ascontiguousarray(b1.reshape(NF, 128).T)
    b2r = np.ascontiguousarray((SW2 * b2).reshape(1, D)).astype(bf16)

    shared = {"utb": utb, "s12": s12, "posn": posn,
              "w1x": w1x, "w1p": w1p, "w2r": w2r, "b1t": b1t, "b2r": b2r}

    in_maps = []
    cls_rows = []
    for b in range(B):
        xb = x[b, 1:]
        yb = y[b, 1:]
        ybp = q8(yb.reshape(4, 2, 128, D).transpose(2, 0, 1, 3))
        xf8 = q8(SY * xb.T.reshape(3, 2, 128, N).transpose(2, 0, 1, 3))
        ybt8 = q8(SY * yb.T.reshape(3, 2, 128, N).transpose(2, 0, 1, 3))
        # CLS output row on host (exact f32 1x1536 MLP; no heat scaling)
        h0 = np.concatenate([x[b, 0], y[b, 0]]) @ W1 + b1
        xp0 = (h0 * 0.5 * (1.0 + erf(h0 / np.sqrt(2.0)))) @ W2 + b2
        cls_rows.append(x[b, 0] + xp0)
        m = dict(shared)
        m["ybp"] = ybp
        m["xf8"] = xf8
        m["ybt8"] = ybt8
        m["xnat"] = np.ascontiguousarray(xb)
        in_maps.append(m)

    res = run_bass_kernel_spmd(nc, in_maps, list(range(B)))
    out = np.empty((B, N + 1, D), np.float32)
    for b in range(B):
        out[b, 0] = cls_rows[b]
        out[b, 1:] = res.results[b]["out"]
    return out


if __name__ == "__main__":
    import time
    sys.path.insert(0, "/root/problem")
    from reference import setup_inputs, reference

    inp = {k: np.asarray(v) for k, v in setup_inputs().items()}
    t0 = time.time()
    got = kernel(**inp)
    print("kernel wall:", time.time() - t0)
    exp = np.asarray(reference(**inp))
    d = np.abs(got - exp)
    print("absmax_rel:", d.max() / np.abs(exp).max())
    print("rms_rel:", np.sqrt((d ** 2).mean()) / np.sqrt((exp ** 2).mean()))
